# revision 40
# baseline (speedup 1.0000x reference)
"""Trainium2 Bass kernel for a DiT-style transformer block (adaLN modulation,
RoPE self-attention with additive rank mask, hybrid cross-attention to
[clean|observed] memory, gated MLP).

Sharding: 8 cores = 4 batches x 2 sequence-halves. Each core computes the
block output for its 512 query tokens of one batch. Per-core token order is
permuted (host side) so the core's own tokens come first, which keeps the
program SPMD-static across cores.

Layout: activations live feature-major ("T-layout", [feature, token]) so all
matmuls contract along partitions. The whole matmul/softmax path runs in
bf16 (weights host-pre-swizzled to [128, blocks, free] so each load is one
contiguous DMA per partition); the residual stream stays fp32 in SBUF
(no DRAM roundtrips between phases). Softmax runs without max-subtraction
(scores are O(10)); masking multiplies probabilities by exp(mask) in {0,1}.
Softmax denominators come free from a ones-column appended to each head's
value block (p@v output row 64). The memory layernorm is folded through the
KV projection (per-token affine commutes with the feature contraction):
kv = rs_t*(W@mem) - (mu*rs)_t*rowsum(W). Overall ~1.18ms (rel err ~2e-3),
from a 1.54ms fp32 baseline.
"""

import numpy as np
import ml_dtypes
from contextlib import ExitStack

BF = ml_dtypes.bfloat16

from concourse import bacc, mybir
import concourse.bass as bass
import concourse.tile as tile
from concourse import bass_utils

F32 = mybir.dt.float32
F32R = mybir.dt.float32r
BF16 = mybir.dt.bfloat16
F8 = mybir.dt.float8e4
F8H = ml_dtypes.float8_e4m3
AF = mybir.ActivationFunctionType
OP = mybir.AluOpType
DR = mybir.MatmulPerfMode.DoubleRow

P = 128
WS = 64.0    # fp8 weight pre-scale (host multiplies weights by WS)
WSI = 1.0 / WS


class Cfg:
    def __init__(self, mini=False):
        if mini:
            self.B, self.N, self.D, self.H, self.HD = 2, 256, 256, 4, 64
            self.COND = 128
        else:
            self.B, self.N, self.D, self.H, self.HD = 4, 1024, 1024, 16, 64
            self.COND = 256
        self.DH = 4 * self.D
        self.SQ = self.N // 2            # own query tokens per core
        self.CH = self.D // P            # d-chunks
        self.HH = self.H * self.HD // P  # head-pair chunks (= H // 2)
        self.KK = self.N // P            # key chunks per N tokens
        self.NF = self.N // self.SQ      # token-free blocks of SQ (=2)
        self.CC = self.COND // P
        self.DHC = self.DH // P
        self.QKK = self.SQ // P          # key chunks per memory quarter
        self.n_cores = 2 * self.B
        self.eps = 1e-5


def _dma_bcast(nc, out_tile, dram_ap, off, n):
    """DMA dram row [1, off:off+n] broadcast to all partitions [P, n]."""
    src = bass.AP(
        tensor=dram_ap.tensor, offset=dram_ap.offset + off, ap=[[0, P], [1, n]]
    )
    nc.gpsimd.dma_start(out=out_tile, in_=src)


def _shift32_dma(nc, dst, src):
    """dst[p] = src[p xor-32 within each 64-block], [128, F] SBUF tiles."""
    for blk in range(2):
        b = blk * 64
        nc.sync.dma_start(out=dst[b : b + 32, :], in_=src[b + 32 : b + 64, :])
        nc.sync.dma_start(out=dst[b + 32 : b + 64, :], in_=src[b : b + 32, :])


def r(ap):
    """fp32 view of an f32r AP for DVE/ACT/gpsimd input reads."""
    return ap.bitcast(F32)


def build_program(cfg: Cfg):
    c = cfg
    nc = bacc.Bacc(
        "TRN2",
        target_bir_lowering=False,
        debug=False,
        enable_asserts=True,
        num_devices=c.n_cores,
    )

    def din(name, shape, dt=F32R):
        return nc.dram_tensor(name, shape, dt, kind="ExternalInput").ap()

    # pre-swizzled (host) bf16 weights/activations: [P, blocks..., free] so
    # every DMA is one contiguous run per partition.
    x_s = din("x_s", [P, c.CH, c.N], BF16)
    xh_s = din("xh_s", [P, c.CH, c.N], BF16)  # (x-mu)*rstd, host-side
    tc_s = din("tc_s", [P, c.CC, c.N], F8)
    hc_s = din("hc_s", [P, c.NF, c.CH, c.SQ], F8)
    ho_s = din("ho_s", [P, c.NF, c.CH, c.SQ], F8)
    wada_s = din("wada_s", [P, 9, c.CH, c.CC, P], F8)
    wqkv_qk = din("wqkv_qk", [P, 2, c.HH, c.CH, P], F8)
    wqkv_v = din("wqkv_v", [P, 2, 4, 2, 512], F8)
    wself_o = din("wself_o", [P, c.CH, c.HH, P], F8)
    wq_c = din("wq_c", [P, c.HH, c.CH, P], F8)
    wkv_k = din("wkv_k", [P, c.HH, c.CH, P], F8)
    wkv_v = din("wkv_v", [P, 2, 4, 2, 512], F8)
    wcross_o = din("wcross_o", [P, c.CH, c.HH, P], F8)
    wm1_s = din("wm1_s", [P, c.DHC, c.CH, P], F8)
    wm2_s = din("wm2_s", [P, 2, c.CH, 16, P], F8)
    mself_s = din("mself_s", [P, c.KK, 2, c.SQ], F8)
    mhc_s = din("mhc_s", [P, c.KK, 2, c.SQ], F8)
    mho_s = din("mho_s", [P, c.KK, 2, c.SQ], F8)
    bada = din("bada", [P, 9 * c.CH], F32)     # pre-scaled by WS
    bada_u = din("bada_u", [P, 9 * c.CH], F32)  # unscaled
    bm1 = din("bm1", [P, c.DHC], F32)
    bm2 = din("bm2", [P, c.CH], F32)
    cqt = din("cqt", [P, c.SQ], BF16)
    sqt = din("sqt", [P, c.SQ], BF16)
    ckts = din("ckts", [P, c.N], BF16)
    skts = din("skts", [P, c.N], BF16)
    cktm = din("cktm", [P, c.N], BF16)
    sktm = din("sktm", [P, c.N], BF16)
    la_self = din("la_self", [1, c.N], BF16)  # rstd per own-order token
    lb_self = din("lb_self", [1, c.N], BF16)  # mean per own-order token
    la_mc = din("la_mc", [1, c.N], BF16)      # rstd, clean memory
    lb_mc = din("lb_mc", [1, c.N], BF16)      # mean*rstd, clean memory
    la_mo = din("la_mo", [1, c.N], BF16)
    lb_mo = din("lb_mo", [1, c.N], BF16)
    swk = din("swk", [P, c.HH], F32)          # rowsum(Wk) per k-feature
    wsumv = din("wsumv", [1, c.H * c.HD], F32)  # rowsum(Wv) per v-feature
    rs_cols = din("rs_cols", [P, 2 * c.KK], F32)    # mem rstd, column layout
    mrs_cols = din("mrs_cols", [P, 2 * c.KK], F32)  # mem mean*rstd, columns
    selb = din("selb", [2 * c.HH, c.HH, P], BF16)   # one-hot pair selector
    selb2 = din("selb2", [2 * c.HH, c.HH, 64], BF16)  # odd-head selector
    ident = din("ident", [P, P], BF16)              # identity (mask inject)
    identd = din("identd", [P, 2, P], F8)           # identity pair (DR inject)
    sh32 = din("sh32", [P, P], BF16)                # xor-32 partition permute
    out_d = nc.dram_tensor("out", [c.D, c.SQ], F32, kind="ExternalOutput").ap()

    with ExitStack() as ctx:
        tc = ctx.enter_context(tile.TileContext(nc))
        persist = ctx.enter_context(tc.tile_pool(name="persist", bufs=1))
        ws = ctx.enter_context(tc.tile_pool(name="wstream", bufs=1))
        tw_pool = ctx.enter_context(tc.tile_pool(name="tw", bufs=5))
        rsp = ctx.enter_context(tc.tile_pool(name="rsp", bufs=1))
        xkv = ctx.enter_context(tc.tile_pool(name="xkv", bufs=1))
        small = ctx.enter_context(tc.tile_pool(name="small", bufs=1))

        def wtile():
            return ws.tile([P, P], BF16, name="wt", tag="wt", bufs=8)

        def wbtile(nk):
            return ws.tile([P, nk, P], BF16, name=f"wb{nk}", tag=f"wb{nk}",
                           bufs=3)


        def tw():
            return tw_pool.tile([P, c.SQ], F32, name="tw", tag="tw")

        def twb():
            return tw_pool.tile([P, c.SQ], BF16, name="twb", tag="twb",
                                bufs=8)

        # ---------- persistent preloads ----------
        TC = persist.tile([P, c.CC, c.N], F8)
        nc.sync.dma_start(out=TC, in_=tc_s)
        CQ = persist.tile([P, c.SQ], BF16)
        nc.sync.dma_start(out=CQ, in_=cqt)
        SQt = persist.tile([P, c.SQ], BF16)
        nc.sync.dma_start(out=SQt, in_=sqt)
        BADA = persist.tile([P, 9 * c.CH], F32)
        nc.sync.dma_start(out=BADA, in_=bada)
        BADAU = persist.tile([P, 9 * c.CH], F32)
        nc.sync.dma_start(out=BADAU, in_=bada_u)
        BM1 = persist.tile([P, c.DHC], F32)
        nc.sync.dma_start(out=BM1, in_=bm1)
        BM2 = persist.tile([P, c.CH], F32)
        nc.sync.dma_start(out=BM2, in_=bm2)

        # SBUF-resident residual stream (x after self-attn / after cross)
        XC1 = persist.tile([P, c.CH, c.SQ], BF16)
        XC2 = persist.tile([P, c.CH, c.SQ], BF16)

        EPS = persist.tile([P, 1], F32)
        nc.vector.memset(EPS, 1e-5)
        ones_f32 = persist.tile([P, 16], F32)
        nc.vector.memset(ones_f32, 1.0)
        ONEB = persist.tile([P, 1], BF16)
        nc.vector.tensor_copy(ONEB, ones_f32[:, 0:1])
        ONES16 = persist.tile([P, 16], BF16)
        nc.vector.tensor_copy(ONES16, ones_f32)
        EXPB = persist.tile([P, 1], F32)
        nc.vector.memset(EXPB, -2.7725887)
        SELB = persist.tile([2 * c.HH, c.HH, P], BF16)
        nc.sync.dma_start(out=SELB, in_=selb)
        SELB2 = persist.tile([2 * c.HH, c.HH, 64], BF16)
        nc.sync.dma_start(out=SELB2, in_=selb2)
        IDENT = persist.tile([P, P], BF16)
        nc.sync.dma_start(out=IDENT, in_=ident)
        IDENTD = persist.tile([P, 2, P], F8)
        nc.sync.dma_start(out=IDENTD, in_=identd)
        SH32 = persist.tile([P, P], BF16)
        nc.sync.dma_start(out=SH32, in_=sh32)

        # ---------- helpers ----------
        def ada_modulate(q_sh, q_sc, x_src, x_nf, la_b, lb_b, xn_dst,
                         xh_src=None):
            """xn = x*sc1 - m*sc1 + sh, with sc1 = rs*w*(1+sc).

            la_b(cols) -> [P, SQ] rstd broadcast AP; lb_b(cols) -> mean.
            x_src(j, tf) / xn_dst(j, tf): [P, SQ] APs.
            """
            with tc.tile_pool(name="ps_ada", bufs=1, space="PSUM") as psa:
                wsh = ws.tile([P, c.CH, c.CC, P], F8, name="wadb",
                              tag="wadb", bufs=2)
                nc.sync.dma_start(out=wsh, in_=wada_s[:, q_sh])
                wsc = ws.tile([P, c.CH, c.CC, P], F8, name="wadb",
                              tag="wadb", bufs=2)
                nc.sync.dma_start(out=wsc, in_=wada_s[:, q_sc])
                for j in range(c.CH):
                    ps_sh = [
                        psa.tile([P, c.SQ], F32, name=f"ps_sh{t}", tag=f"ps_sh{t}")
                        for t in range(x_nf)
                    ]
                    ps_sc = [
                        psa.tile([P, c.SQ], F32, name=f"ps_sc{t}", tag=f"ps_sc{t}")
                        for t in range(x_nf)
                    ]
                    for tf in range(x_nf):
                        nc.tensor.matmul(
                            ps_sh[tf], wsh[:, j],
                            TC[:, :, tf * c.SQ : (tf + 1) * c.SQ],
                            start=True, stop=True, perf_mode=DR,
                        )
                    for tf in range(x_nf):
                        nc.tensor.matmul(
                            ps_sc[tf], wsc[:, j],
                            TC[:, :, tf * c.SQ : (tf + 1) * c.SQ],
                            start=True, stop=True, perf_mode=DR,
                        )
                    for tf in range(x_nf):
                        cols = slice(tf * c.SQ, (tf + 1) * c.SQ)
                        # xn = (x - m)*(yc*la) + sh0. Both psum evictions
                        # ride ACT (idle here); DVE ops stay all-bf16 SBUF
                        # so they hit the 2x DVE rate; xd runs on GpSimd.
                        yc = twb()
                        nc.scalar.activation(
                            yc, ps_sc[tf], AF.Identity, scale=WSI,
                            bias=BADAU[:, q_sc * c.CH + j : q_sc * c.CH + j + 1],
                        )
                        sh0 = twb()
                        nc.scalar.activation(
                            sh0, ps_sh[tf], AF.Identity, scale=WSI,
                            bias=BADAU[:, q_sh * c.CH + j : q_sh * c.CH + j + 1],
                        )
                        if xh_src is not None:
                            t = twb()
                            nc.vector.tensor_mul(t, xh_src(j, tf), yc)
                        else:
                            xd = twb()
                            nc.vector.tensor_sub(
                                xd, x_src(j, tf), lb_b(cols)
                            )
                            sc1 = twb()
                            nc.vector.tensor_mul(sc1, yc, la_b(cols))
                            t = twb()
                            nc.vector.tensor_mul(t, xd, sc1)
                        nc.vector.tensor_add(xn_dst(j, tf), t, sh0)

        def gate_wtile(q_g):
            wg = ws.tile([P, c.CH, c.CC, P], F8, name="wadb", tag="wadb",
                         bufs=2)
            nc.sync.dma_start(out=wg, in_=wada_s[:, q_g])
            return wg

        def ada_gate_one(q_g, wg, j, psg, unscale):
            """Return a [P, SQ] f32 tile holding gate chunk j on demand.

            unscale folds both the gate's own fp8 weight scale and the
            downstream psum's weight scale into the gate values."""
            ps = psg.tile([P, c.SQ], F32, name="ps_g", tag="ps_g")
            nc.tensor.matmul(
                ps, wg[:, j], TC[:, :, 0 : c.SQ],
                start=True, stop=True, perf_mode=DR,
            )
            g = tw()
            nc.vector.tensor_scalar(
                g, ps, BADA[:, q_g * c.CH + j : q_g * c.CH + j + 1],
                unscale, op0=OP.add, op1=OP.mult,
            )
            return g

        def rope_evict(zsrc, hh, cols_t, ctab, stab, dst, psp):
            """dst[:, hh, cols_t] = zsrc*cos + shift32(zsrc)*sin_signed.

            The xor-32 partition shift runs on the PE (permutation matmul)
            instead of 4 small SBUF-to-SBUF DMAs."""
            ps2 = psp.tile([P, c.SQ], F32, name="ps_sh32", tag="ps_sh32",
                           bufs=2)
            nc.tensor.matmul(ps2, SH32, zsrc, start=True, stop=True)
            t1 = twb()
            nc.vector.tensor_mul(t1, zsrc, ctab)
            t2 = twb()
            nc.vector.tensor_mul(t2, ps2, stab)
            nc.vector.tensor_add(dst[:, hh, cols_t], t1, t2)

        def proj_rope(wsel, n_free, ctab, stab, dst, src_tile):
            """dst[:, hh, :] = rope(W[:, cols].T @ src), head-pair chunks."""
            nf = n_free // c.SQ
            with tc.tile_pool(name="ps_qk", bufs=3, space="PSUM") as psq:
                wt = ws.tile([P, c.HH, c.CH, P], F8, name="wpr", tag="wpr",
                             bufs=1)
                nc.sync.dma_start(out=wt, in_=wsel)
                for hh in range(c.HH):
                    for tf in range(nf):
                        ps = psq.tile([P, c.SQ], F32, name="ps_qk", tag="ps_qk")
                        for kp in range(c.CH // 2):
                            nc.tensor.matmul(
                                ps, wt[:, hh, 2 * kp : 2 * kp + 2, :],
                                src_tile[:, 2 * kp : 2 * kp + 2,
                                         tf * c.SQ : (tf + 1) * c.SQ],
                                start=(kp == 0), stop=(kp == c.CH // 2 - 1),
                                perf_mode=DR,
                            )
                        cols = slice(tf * c.SQ, (tf + 1) * c.SQ)
                        traw = twb()
                        nc.scalar.activation(traw, ps, AF.Copy, scale=WSI)
                        rope_evict(
                            traw, hh, cols, ctab[:, cols], stab[:, cols],
                            dst, psq,
                        )

        def vproj_self(src_tile, vdst, wvp):
            """Token-major value projection from resident XN; ones cols."""
            ntt = c.KK
            ffw = min(512, c.H * c.HD)
            nff = (c.H * c.HD) // ffw
            hpf = ffw // 64
            for tt in range(ntt):
                ap = vdst[:, tt, :].rearrange("p (h e) -> p h e", e=65)[:, :, 64:65]
                nc.vector.tensor_copy(ap, ONES16[:, 0 : c.H])
            wt_all = wvp.tile([P, 2, 4, 2, ffw], F8, name="wv", tag="wv",
                              bufs=1)
            nc.sync.dma_start(out=wt_all, in_=wqkv_v)
            with tc.tile_pool(name="ps_v", bufs=8, space="PSUM") as psv:
                for ff in range(nff):
                    pss = [
                        psv.tile([P, ffw], F32, name="ps_v", tag="ps_v")
                        for _ in range(ntt)
                    ]
                    for kp in range(c.CH // 2):
                        for tt in range(ntt):
                            nc.tensor.matmul(
                                pss[tt],
                                src_tile[:, 2 * kp : 2 * kp + 2,
                                         tt * P : (tt + 1) * P],
                                wt_all[:, ff, kp, :, :],
                                start=(kp == 0), stop=(kp == c.CH // 2 - 1),
                                perf_mode=DR,
                            )
                    for tt in range(ntt):
                        ap = (
                            vdst[:, tt, ff * hpf * 65 : (ff + 1) * hpf * 65]
                            .rearrange("p (h e) -> p h e", e=65)[:, :, 0:64]
                        )
                        nc.vector.tensor_scalar_mul(ap, pss[tt], WSI)

        def attention_hp(hp, khat, vtile, qhat, masks, n_kk, ps_o1, ps_o2,
                         tp_pool, first, last, pss_bufs=3):
            """One head pair, software-pipelined: p@v (fp8 DoubleRow over
            key-chunk pairs) lags scores by one pair so the PE has
            independent work while ACT runs exp on the current pair."""
            h1, h2 = 2 * hp, 2 * hp + 1
            npair = n_kk // 2

            def pv(pc, pt):
                nc.tensor.matmul(
                    ps_o1,
                    vtile[:, 2 * pc : 2 * pc + 2, h1 * 65 : (h1 + 1) * 65],
                    pt[:, :, 0 : c.SQ],
                    start=(first and pc == 0),
                    stop=(last and pc == npair - 1),
                    perf_mode=DR,
                )
                nc.tensor.matmul(
                    ps_o2,
                    vtile[:, 2 * pc : 2 * pc + 2, h2 * 65 : (h2 + 1) * 65],
                    pt[:, :, c.SQ : 2 * c.SQ],
                    start=(first and pc == 0),
                    stop=(last and pc == npair - 1),
                    perf_mode=DR,
                )

            with tc.tile_pool(name="ps_s", bufs=pss_bufs,
                              space="PSUM") as pss:
                prev = None
                for pc in range(npair):
                    pt = tp_pool.tile(
                        [P, 2, 2 * c.SQ], F8, name="t_p", tag="t_p", bufs=2
                    )
                    for j in range(2):
                        kkc = 2 * pc + j
                        ps = pss.tile([P, 2 * c.SQ], F32, name="ps_s",
                                      tag="ps_s")
                        ks = slice(kkc * P, (kkc + 1) * P)
                        # additive mask injected into PSUM via identity
                        # matmul, then scores accumulate on top;
                        # exp(scores+mask) needs no mask multiply after.
                        nc.tensor.matmul(
                            ps[:, 0 : c.SQ], IDENTD, masks[:, kkc],
                            start=True, stop=False, perf_mode=DR,
                        )
                        nc.tensor.matmul(
                            ps[:, c.SQ : 2 * c.SQ], IDENTD, masks[:, kkc],
                            start=True, stop=False, perf_mode=DR,
                        )
                        nc.tensor.matmul(
                            ps[:, 0 : c.SQ],
                            khat[0:64, hp, ks], qhat[0:64, hp, :],
                            start=False, stop=True,
                        )
                        nc.tensor.matmul(
                            ps[:, c.SQ : 2 * c.SQ],
                            khat[64:128, hp, ks], qhat[64:128, hp, :],
                            start=False, stop=True,
                        )
                        # exp(s - ln 16): keeps fp8 probs < 240; the 1/16
                        # cancels in the softmax normalization.
                        nc.scalar.activation(pt[:, j, :], ps, AF.Exp,
                                             bias=EXPB[:, 0:1])
                    if prev is not None:
                        pv(*prev)
                    prev = (pc, pt)
                pv(*prev)

        def evict_unnorm(ps_o, hp, second, odst, den, tp_pool):
            """Stage unnormalized o rows into odst and the denominator row
            into den[2hp+second]. Normalization happens batched later."""
            h = 2 * hp + (1 if second else 0)
            dstage = tp_pool.tile(
                [65, c.SQ], F32, name="t_dstage", tag="t_dstage", bufs=2
            )
            nc.vector.tensor_copy(dstage[64:65, :], ps_o[64:65, :])
            nc.sync.dma_start(out=den[h : h + 1, :], in_=dstage[64:65, :])
            if not second:
                nc.vector.tensor_copy(odst[0:64, hp, :], ps_o[0:64, :])
            else:
                st = tp_pool.tile(
                    [64, c.SQ], BF16, name="t_onorm", tag="t_onorm", bufs=2
                )
                nc.vector.tensor_copy(st, ps_o[0:64, :])
                nc.sync.dma_start(out=odst[64:128, hp, :], in_=st)

        def normalize_batch(osrc_u, odst, den, deni, tp_pool, n_hp):
            """odst[:, hp, :] = osrc_u * 1/den rows (PE one-hot broadcast)."""
            with nc.allow_low_precision(reason="bf16 softmax denominators"):
                nc.vector.reciprocal(deni, den)
            with tc.tile_pool(name="ps_nb", bufs=2, space="PSUM") as pnb:
                for hp in range(n_hp):
                    ps_rb = pnb.tile([P, c.SQ], F32, name="ps_rb",
                                     tag="ps_rb")
                    nc.tensor.matmul(
                        ps_rb, SELB[:, hp, :], deni, start=True, stop=True
                    )
                    nc.vector.tensor_mul(
                        odst[:, hp, :], osrc_u[:, hp, :], ps_rb
                    )

        def out_proj_residual(wo_s, osrc, g_src, xr, xdst, stats=None):
            with tc.tile_pool(name="ps_op", bufs=3, space="PSUM") as pso:
                wt = ws.tile([P, c.CH, c.HH, P], F8, name="wop", tag="wop",
                             bufs=1)
                nc.sync.dma_start(out=wt, in_=wo_s)
                for j in range(c.CH):
                    ps = pso.tile([P, c.SQ], F32, name="ps_op", tag="ps_op")
                    for hp in range(c.HH // 2):
                        nc.tensor.matmul(
                            ps, wt[:, j, 2 * hp : 2 * hp + 2, :],
                            osrc[:, 2 * hp : 2 * hp + 2, :],
                            start=(hp == 0), stop=(hp == c.HH // 2 - 1),
                            perf_mode=DR,
                        )
                    t = tw()
                    nc.vector.tensor_mul(t, ps, g_src(j))
                    nc.vector.tensor_add(xdst(j), t, xr(j))

        def ln_stats_make(psst, stp):
            return {
                "ps1": psst.tile([1, c.SQ], F32, name="ps_st1", tag="ps_st1"),
                "ps2": psst.tile([1, c.SQ], F32, name="ps_st2", tag="ps_st2"),
                "stp": stp,
            }

        def ln_stats_accum(st, j, xa):
            sq = st["stp"].tile([P, c.SQ], BF16, name="t_sq", tag="t_sq",
                                bufs=2)
            nc.vector.tensor_mul(sq, xa, xa)
            nc.tensor.matmul(
                st["ps1"], ONEB, xa, start=(j == 0), stop=(j == c.CH - 1)
            )
            nc.tensor.matmul(
                st["ps2"], ONEB, sq, start=(j == 0), stop=(j == c.CH - 1)
            )

        def ln_stats_finish(st):
            """[P, SQ] broadcast (rstd, mean) bf16 tiles from the psums."""
            rs_b = rsp.tile([P, c.SQ], BF16, name="t_rsb", tag="t_rsb")
            m_b = rsp.tile([P, c.SQ], BF16, name="t_mb", tag="t_mb")
            stp = st["stp"]
            m = stp.tile([1, c.SQ], F32, name="s_m", tag="s_m")
            nc.vector.tensor_scalar_mul(m, st["ps1"][0:1, :], 1.0 / c.D)
            e2 = stp.tile([1, c.SQ], F32, name="s_a", tag="s_a")
            nc.vector.tensor_scalar_mul(e2, st["ps2"][0:1, :], 1.0 / c.D)
            msq = stp.tile([1, c.SQ], F32, name="s_b", tag="s_b")
            nc.vector.tensor_mul(msq, m, m)
            var = stp.tile([1, c.SQ], F32, name="s_c", tag="s_c")
            nc.vector.tensor_sub(var, e2, msq)
            sd = stp.tile([1, c.SQ], F32, name="s_d", tag="s_d")
            nc.scalar.activation(sd, var, AF.Sqrt, bias=EPS[0:1, :])
            rs = stp.tile([1, c.SQ], F32, name="s_e", tag="s_e")
            nc.vector.reciprocal(rs, sd)
            rs16 = stp.tile([1, c.SQ], BF16, name="s_e16", tag="s_e16")
            nc.vector.tensor_copy(rs16, rs)
            m16 = stp.tile([1, c.SQ], BF16, name="s_m16", tag="s_m16")
            nc.vector.tensor_copy(m16, m)
            nc.gpsimd.partition_broadcast(rs_b, rs16, channels=P)
            nc.gpsimd.partition_broadcast(m_b, m16, channels=P)
            return rs_b, m_b

        def stream_x(dram, j, cols):
            t = tw()
            nc.sync.dma_start(out=t, in_=r(dram[j * P : (j + 1) * P, cols]))
            return t

        def stream_xr(dram, j):
            t = tw_pool.tile([P, c.SQ], F32R, name="twr", tag="twr", bufs=2)
            nc.sync.dma_start(out=t, in_=dram[j * P : (j + 1) * P, :])
            return t

        KCF = xkv.tile([P, c.HH, 2 * c.N], BF16)
        VCF = xkv.tile([P, 2 * c.KK, c.H * 65], F8)

        def build_cross_kv():
            """Build the full cross-attention K (rope'd, bf16) and V (fp8)
            from the memory streams; independent of phase 1, emitted so its
            DVE work overlaps phase-1 attention."""
            ffw = min(512, c.H * c.HD)
            nff = (c.H * c.HD) // ffw
            hpf = ffw // 64
            with tc.tile_pool(name="p2h", bufs=1) as p2h, \
                 tc.tile_pool(name="mstr", bufs=1) as mstr:
              SWK = p2h.tile([P, c.HH], F32)
              nc.sync.dma_start(out=SWK, in_=swk)
              WSVb = p2h.tile([P, c.H * c.HD], F32)
              _dma_bcast(nc, WSVb, wsumv, 0, c.H * c.HD)
              RSC = p2h.tile([P, 2 * c.KK], F32)
              nc.sync.dma_start(out=RSC, in_=rs_cols)
              MRSC = p2h.tile([P, 2 * c.KK], F32)
              nc.sync.dma_start(out=MRSC, in_=mrs_cols)
              WKV_K = p2h.tile([P, c.HH, c.CH, P], F8)
              nc.sync.dma_start(out=WKV_K, in_=wkv_k)
              WKV_V = p2h.tile([P, 2, 4, 2, 512], F8)
              nc.sync.dma_start(out=WKV_V, in_=wkv_v)
              for qq in range(2 * c.NF):
                half = qq // c.NF            # 0: clean, 1: observed
                hq = qq % c.NF               # quarter index within half
                mem_s = hc_s if half == 0 else ho_s
                la_m = la_mc if half == 0 else la_mo
                lb_m = lb_mc if half == 0 else lb_mo
                tok0 = hq * c.SQ
                ctok = slice(tok0, tok0 + c.SQ)

                MEMQ = p2h.tile([P, c.CH, c.SQ], F8, name="MEMQ",
                                tag="MEMQ", bufs=2)
                nc.sync.dma_start(out=MEMQ, in_=mem_s[:, hq])
                CKm_t = p2h.tile([P, c.SQ], BF16, name="CKm", tag="CKm",
                                 bufs=2)
                nc.sync.dma_start(out=CKm_t, in_=cktm[:, ctok])
                SKm_t = p2h.tile([P, c.SQ], BF16, name="SKm", tag="SKm",
                                 bufs=2)
                nc.sync.dma_start(out=SKm_t, in_=sktm[:, ctok])
                LAm = p2h.tile([P, c.SQ], BF16, name="LAm", tag="LAm",
                               bufs=2)
                _dma_bcast(nc, LAm, la_m, tok0, c.SQ)
                LBm = p2h.tile([P, c.SQ], BF16, name="LBm", tag="LBm",
                               bufs=2)
                _dma_bcast(nc, LBm, lb_m, tok0, c.SQ)

                with tc.tile_pool(name="ps_kp", bufs=2, space="PSUM") as pkp:
                    for hh in range(c.HH):
                        pk = pkp.tile([P, c.SQ], F32, name="ps_k",
                                      tag="ps_k")
                        for kp in range(c.CH // 2):
                            nc.tensor.matmul(
                                pk, WKV_K[:, hh, 2 * kp : 2 * kp + 2, :],
                                MEMQ[:, 2 * kp : 2 * kp + 2, :],
                                start=(kp == 0), stop=(kp == c.CH // 2 - 1),
                                perf_mode=DR,
                            )
                        # LN fold: z = ps*rs_t - (mu*rs)_t * rowsum(Wk)
                        t2 = twb()
                        nc.vector.tensor_scalar_mul(
                            t2, LBm, SWK[:, hh : hh + 1]
                        )
                        t1 = twb()
                        nc.vector.tensor_mul(t1, pk, LAm)
                        z = twb()
                        nc.vector.tensor_sub(z, t1, t2)
                        rope_evict(
                            z, hh, slice(qq * c.SQ, (qq + 1) * c.SQ),
                            CKm_t, SKm_t, KCF, pkp,
                        )

                for tt in range(c.QKK):
                    ap = VCF[:, qq * c.QKK + tt, :].rearrange(
                        "p (h e) -> p h e", e=65
                    )[:, :, 64:65]
                    nc.vector.tensor_copy(ap, ONES16[:, 0 : c.H])
                with tc.tile_pool(name="ps_v2", bufs=4, space="PSUM") as psv:
                    for ff in range(nff):
                        pss = [
                            psv.tile([P, ffw], F32, name="ps_v2",
                                     tag="ps_v2")
                            for _ in range(c.QKK)
                        ]
                        for kp in range(c.CH // 2):
                            for tt in range(c.QKK):
                                nc.tensor.matmul(
                                    pss[tt],
                                    MEMQ[:, 2 * kp : 2 * kp + 2,
                                         tt * P : (tt + 1) * P],
                                    WKV_V[:, ff, kp, :, :],
                                    start=(kp == 0),
                                    stop=(kp == c.CH // 2 - 1),
                                    perf_mode=DR,
                                )
                        for tt in range(c.QKK):
                            tok_col = half * c.KK + hq * c.QKK + tt
                            t2 = mstr.tile(
                                [P, ffw], F32, name="tvw", tag="tvw",
                                bufs=2,
                            )
                            nc.vector.tensor_scalar_mul(
                                t2, WSVb[:, ff * ffw : (ff + 1) * ffw],
                                MRSC[:, tok_col : tok_col + 1],
                            )
                            ap = VCF[
                                :, qq * c.QKK + tt,
                                ff * hpf * 65 : (ff + 1) * hpf * 65
                            ].rearrange("p (h e) -> p h e", e=65)[:, :, 0:64]
                            nc.vector.scalar_tensor_tensor(
                                out=ap, in0=pss[tt],
                                scalar=RSC[:, tok_col : tok_col + 1],
                                in1=t2, op0=OP.mult, op1=OP.subtract,
                            )


        # =======================================================
        # Phase 1: self-attention
        # =======================================================
        with tc.tile_pool(name="p1", bufs=1) as p1:
            QHAT = p1.tile([P, c.HH, c.SQ], BF16)
            KHAT = p1.tile([P, c.HH, c.N], BF16)
            VSELF = p1.tile([P, c.KK, c.H * 65], F8)
            X0 = p1.tile([P, c.CH, c.SQ], BF16)
            for jj in range(c.CH):
                nc.sync.dma_start(out=X0[:, jj], in_=x_s[:, jj, 0 : c.SQ])

            with tc.tile_pool(name="p1a", bufs=1) as p1a:
                XN = p1a.tile([P, c.CH, c.N], F8)
                XH = p1a.tile([P, c.CH, c.N], BF16)
                for jj in range(c.CH):
                    nc.sync.dma_start(out=XH[:, jj], in_=xh_s[:, jj])
                CKs_t = p1a.tile([P, c.N], BF16)
                nc.sync.dma_start(out=CKs_t, in_=ckts)
                SKs_t = p1a.tile([P, c.N], BF16)
                nc.sync.dma_start(out=SKs_t, in_=skts)
                with tc.tile_pool(name="p1ln", bufs=1) as p1ln:
                    ada_modulate(
                        0, 1,
                        None,
                        c.NF,
                        None,
                        None,
                        lambda j, tf: XN[:, j, tf * c.SQ : (tf + 1) * c.SQ],
                        xh_src=lambda j, tf: XH[
                            :, j, tf * c.SQ : (tf + 1) * c.SQ
                        ],
                    )
                proj_rope(wqkv_qk[:, 0], c.SQ, CQ, SQt, QHAT, XN)
                proj_rope(wqkv_qk[:, 1], c.N, CKs_t, SKs_t, KHAT, XN)
                with tc.tile_pool(name="wvp1", bufs=1) as wvp:
                    vproj_self(XN, VSELF, wvp)

            build_cross_kv()

            with tc.tile_pool(name="p1b", bufs=1) as p1b, \
                 tc.tile_pool(name="tp1", bufs=1) as tp1:
                MS = p1b.tile([P, c.KK, 2, c.SQ], F8)
                nc.sync.dma_start(out=MS, in_=mself_s)
                OSELFU = p1b.tile([P, c.HH, c.SQ], BF16)
                OSELF = p1b.tile([P, c.HH, c.SQ], F8)

                DENS = p1b.tile([2 * c.HH, c.SQ], F32)
                DENSI = p1b.tile([2 * c.HH, c.SQ], BF16)
                with tc.tile_pool(name="ps_oacc", bufs=2, space="PSUM") as psoa:
                    for hp in range(c.HH):
                        ps_o1 = psoa.tile(
                            [65, c.SQ], F32, name="ps_o1", tag="ps_o1"
                        )
                        ps_o2 = psoa.tile(
                            [65, c.SQ], F32, name="ps_o2", tag="ps_o2"
                        )
                        attention_hp(
                            hp, KHAT, VSELF, QHAT, MS, c.KK,
                            ps_o1, ps_o2, tp1, True, True, pss_bufs=2,
                        )
                        evict_unnorm(ps_o1, hp, False, OSELFU, DENS, tp1)
                        evict_unnorm(ps_o2, hp, True, OSELFU, DENS, tp1)
                normalize_batch(OSELFU, OSELF, DENS, DENSI, tp1, c.HH)

                with tc.tile_pool(name="ps_gx", bufs=2, space="PSUM") as psg:
                    wg1 = gate_wtile(2)
                    out_proj_residual(
                        wself_o, OSELF,
                        lambda j: ada_gate_one(2, wg1, j, psg, WSI * WSI),
                        lambda j: X0[:, j, :],
                        lambda j: XC1[:, j, :],
                    )

        # =======================================================
        # Phase 2: cross-attention (memory quarters, LN folded into proj)
        # =======================================================
        with tc.tile_pool(name="p2", bufs=1) as p2:
            with tc.tile_pool(name="ps_st", bufs=1, space="PSUM") as psst, \
                 tc.tile_pool(name="stats", bufs=1) as stp:
                st1 = ln_stats_make(psst, stp)
                for j in range(c.CH):
                    ln_stats_accum(st1, j, XC1[:, j, :])
                rs_b, m_b = ln_stats_finish(st1)
            QC = p2.tile([P, c.HH, c.SQ], BF16)
            with tc.tile_pool(name="p2q", bufs=1) as p2q:
                XNC = p2q.tile([P, c.CH, c.SQ], F8)
                ada_modulate(
                    3, 4, lambda j, tf: XC1[:, j, :], 1,
                    lambda cols: rs_b[:, cols], lambda cols: m_b[:, cols],
                    lambda j, tf: XNC[:, j, :],
                )
                proj_rope(wq_c, c.SQ, CQ, SQt, QC, XNC)

            MKF = p2.tile([P, 2 * c.KK, 2, c.SQ], F8)
            nc.sync.dma_start(out=MKF[:, 0 : c.KK], in_=mhc_s)
            nc.sync.dma_start(out=MKF[:, c.KK :], in_=mho_s)

            with tc.tile_pool(name="p2b", bufs=1) as p2b, \
                 tc.tile_pool(name="tp2", bufs=1) as tp2:
                OCU = p2b.tile([P, c.HH, c.SQ], BF16)
                OC = p2b.tile([P, c.HH, c.SQ], F8)
                DENC = p2b.tile([2 * c.HH, c.SQ], F32)
                DENCI = p2b.tile([2 * c.HH, c.SQ], BF16)
                with tc.tile_pool(name="ps_oc", bufs=2, space="PSUM") as psoc:
                    for hp in range(c.HH):
                        ps_o1 = psoc.tile(
                            [65, c.SQ], F32, name="ps_oc1", tag="ps_oc1"
                        )
                        ps_o2 = psoc.tile(
                            [65, c.SQ], F32, name="ps_oc2", tag="ps_oc2"
                        )
                        attention_hp(
                            hp, KCF, VCF, QC, MKF, 2 * c.KK,
                            ps_o1, ps_o2, tp2, True, True, pss_bufs=2,
                        )
                        evict_unnorm(ps_o1, hp, False, OCU, DENC, tp2)
                        evict_unnorm(ps_o2, hp, True, OCU, DENC, tp2)
                normalize_batch(OCU, OC, DENC, DENCI, tp2, c.HH)
                with tc.tile_pool(name="ps_gx", bufs=2, space="PSUM") as psg:
                    wg2 = gate_wtile(5)
                    out_proj_residual(
                        wcross_o, OC,
                        lambda j: ada_gate_one(5, wg2, j, psg, WSI * WSI),
                        lambda j: XC1[:, j, :],
                        lambda j: XC2[:, j, :],
                    )

        # =======================================================
        # Phase 3: MLP (two hidden halves, SBUF accumulation)
        # =======================================================
        with tc.tile_pool(name="p3", bufs=1) as p3:
            with tc.tile_pool(name="ps_st", bufs=1, space="PSUM") as psst, \
                 tc.tile_pool(name="stats", bufs=1) as stp:
                st2 = ln_stats_make(psst, stp)
                for j in range(c.CH):
                    ln_stats_accum(st2, j, XC2[:, j, :])
                rs_b, m_b = ln_stats_finish(st2)
            OUT_ACC = p3.tile([P, c.CH, c.SQ], F32)
            G3 = p3.tile([P, c.CH, c.SQ], BF16)

            with tc.tile_pool(name="p3x", bufs=1) as p3x:
                XNM = p3x.tile([P, c.CH, c.SQ], F8)
                ada_modulate(
                    6, 7, lambda j, tf: XC2[:, j, :], 1,
                    lambda cols: rs_b[:, cols], lambda cols: m_b[:, cols],
                    lambda j, tf: XNM[:, j, :],
                )
                with tc.tile_pool(name="ps_gx", bufs=2, space="PSUM") as psg:
                    wg3 = gate_wtile(8)
                    for j in range(c.CH):
                        ps = psg.tile([P, c.SQ], F32, name="ps_g", tag="ps_g")
                        nc.tensor.matmul(
                            ps, wg3[:, j], TC[:, :, 0 : c.SQ],
                            start=True, stop=True, perf_mode=DR,
                        )
                        nc.vector.tensor_scalar(
                            G3[:, j, :], ps,
                            BADA[:, 8 * c.CH + j : 8 * c.CH + j + 1],
                            WSI, op0=OP.add, op1=OP.mult,
                        )
                outv = out_d.rearrange("(k p) q -> p k q", p=P)
                nhalf = c.DHC // 2
                # streamed weight chunks + disjoint psum pools for m1/m2 so
                # half boundaries pipeline instead of serializing on 4MB DMAs
                with tc.tile_pool(name="p3h", bufs=1) as p3h, \
                     tc.tile_pool(name="ps_m1", bufs=3, space="PSUM") as psm, \
                     tc.tile_pool(name="ps_m2", bufs=3, space="PSUM") as psm2:
                    for half in range(2):
                        HT = p3h.tile([P, nhalf, c.SQ], F8, name="HT",
                                      tag="HT", bufs=2)
                        for jg in range(2):
                            wm1t = p3h.tile([P, 8, c.CH, P], F8,
                                            name="wm1c", tag="wm1c", bufs=2)
                            j0 = half * nhalf + jg * 8
                            nc.sync.dma_start(
                                out=wm1t, in_=wm1_s[:, j0 : j0 + 8]
                            )
                            for j8 in range(8):
                                jj = jg * 8 + j8
                                gj = half * nhalf + jj
                                ps = psm.tile(
                                    [P, c.SQ], F32, name="ps_m1", tag="ps_m1"
                                )
                                for kp in range(c.CH // 2):
                                    nc.tensor.matmul(
                                        ps, wm1t[:, j8, 2 * kp : 2 * kp + 2, :],
                                        XNM[:, 2 * kp : 2 * kp + 2, :],
                                        start=(kp == 0),
                                        stop=(kp == c.CH // 2 - 1),
                                        perf_mode=DR,
                                    )
                                nc.scalar.activation(
                                    HT[:, jj, :], ps, AF.Gelu_apprx_tanh,
                                    bias=BM1[:, gj : gj + 1], scale=WSI,
                                )
                        for jg in range(4):
                            wm2t = p3h.tile([P, 2, nhalf, P], F8,
                                            name="wm2c", tag="wm2c", bufs=2)
                            nc.sync.dma_start(
                                out=wm2t,
                                in_=wm2_s[:, half, jg * 2 : (jg + 1) * 2],
                            )
                            for j2 in range(2):
                                j = jg * 2 + j2
                                ps = psm2.tile(
                                    [P, c.SQ], F32, name="ps_m2", tag="ps_m2"
                                )
                                for kp in range(nhalf // 2):
                                    nc.tensor.matmul(
                                        ps, wm2t[:, j2, 2 * kp : 2 * kp + 2, :],
                                        HT[:, 2 * kp : 2 * kp + 2, :],
                                        start=(kp == 0),
                                        stop=(kp == nhalf // 2 - 1),
                                        perf_mode=DR,
                                    )
                                if half == 0:
                                    nc.vector.tensor_scalar_mul(
                                        OUT_ACC[:, j, :], ps, WSI
                                    )
                                else:
                                    s1 = tw()
                                    nc.vector.scalar_tensor_tensor(
                                        out=s1, in0=ps,
                                        scalar=WSI, in1=OUT_ACC[:, j, :],
                                        op0=OP.mult, op1=OP.add,
                                    )
                                    t = tw()
                                    nc.vector.scalar_tensor_tensor(
                                        out=t, in0=s1,
                                        scalar=BM2[:, j : j + 1],
                                        in1=G3[:, j, :],
                                        op0=OP.add, op1=OP.mult,
                                    )
                                    nc.vector.tensor_add(
                                        OUT_ACC[:, j, :], t, XC2[:, j, :]
                                    )
                                    nc.sync.dma_start(
                                        out=outv[:, j, :],
                                        in_=OUT_ACC[:, j, :],
                                    )

    nc.compile()
    return nc


# =======================================================
# Host side
# =======================================================

def host_prep(cfg: Cfg, inputs: dict):
    c = cfg
    f32 = np.float32

    q_x = np.asarray(inputs["q_x"], f32)
    h_content = np.asarray(inputs["h_content"], f32)
    h_obs = np.asarray(inputs["h_obs"], f32)
    t_cond = np.asarray(inputs["t_cond"], f32)
    M_QQ = np.asarray(inputs["M_QQ"], f32)
    M_hyb = np.asarray(inputs["M_hyb"], f32)
    w_ln_self = np.asarray(inputs["w_ln_self"], f32)
    w_qkv = np.asarray(inputs["w_qkv"], f32)
    w_self_out = np.asarray(inputs["w_self_out"], f32)
    w_ln_cross = np.asarray(inputs["w_ln_cross"], f32)
    w_ln_mem = np.asarray(inputs["w_ln_mem"], f32)
    w_qproj = np.asarray(inputs["w_qproj"], f32)
    w_kvproj = np.asarray(inputs["w_kvproj"], f32)
    w_cross_out = np.asarray(inputs["w_cross_out"], f32)
    w_ln_mlp = np.asarray(inputs["w_ln_mlp"], f32)
    w_mlp1 = np.asarray(inputs["w_mlp1"], f32)
    b_mlp1 = np.asarray(inputs["b_mlp1"], f32)
    w_mlp2 = np.asarray(inputs["w_mlp2"], f32)
    b_mlp2 = np.asarray(inputs["b_mlp2"], f32)
    w_ada = np.asarray(inputs["w_ada"], f32)
    b_ada = np.asarray(inputs["b_ada"], f32)

    D, N, HD, SQ = c.D, c.N, c.HD, c.SQ

    wada9 = w_ada[: 9 * D].copy()
    bada9 = b_ada[: 9 * D].copy()
    for q, wl in ((1, w_ln_self), (4, w_ln_cross), (7, w_ln_mlp)):
        wada9[q * D : (q + 1) * D] *= wl[:, None]
        bada9[q * D : (q + 1) * D] = wl * (1.0 + b_ada[q * D : (q + 1) * D])
    wadaT = np.ascontiguousarray(wada9.T)
    bada_u_h = np.ascontiguousarray(bada9.reshape(9 * c.CH, P).T)
    bada_h = np.ascontiguousarray(bada_u_h * WS)

    def bfc(a):
        return np.ascontiguousarray(a, dtype=BF)

    def f8c(a):
        return np.ascontiguousarray(
            np.clip(np.asarray(a, f32) * WS, -240.0, 240.0), dtype=F8H
        )

    def f8a(a):
        return np.ascontiguousarray(
            np.clip(np.asarray(a, f32), -240.0, 240.0), dtype=F8H
        )

    wqkvT = np.ascontiguousarray(w_qkv.T)
    wkv_eff = w_kvproj * w_ln_mem[None, :]
    wkvT = np.ascontiguousarray(wkv_eff.T)
    # pre-swizzled bf16 weight layouts ([P, blocks..., free])
    wada_s_h = f8c(wadaT.reshape(2, P, 9, 8, P).transpose(1, 2, 3, 0, 4))
    wqkv_qk_h = f8c(
        wqkvT[:, : 2 * D].reshape(8, P, 2, 8, P).transpose(1, 2, 3, 0, 4)
    )
    wqkv_v_h = f8c(
        wqkvT[:, 2 * D :].reshape(4, 2, P, 2, 512).transpose(2, 3, 0, 1, 4)
    )
    wself_o_h = f8c(w_self_out.T.reshape(8, P, 8, P).transpose(1, 2, 0, 3))
    wq_c_h = f8c(w_qproj.T.reshape(8, P, 8, P).transpose(1, 2, 0, 3))
    wkv_k_h = f8c(wkvT[:, :D].reshape(8, P, 8, P).transpose(1, 2, 0, 3))
    wkv_v_h = f8c(
        wkvT[:, D:].reshape(4, 2, P, 2, 512).transpose(2, 3, 0, 1, 4)
    )
    wcross_o_h = f8c(w_cross_out.T.reshape(8, P, 8, P).transpose(1, 2, 0, 3))
    wm1_s_h = f8c(w_mlp1.T.reshape(8, P, 32, P).transpose(1, 2, 0, 3))
    wm2_s_h = f8c(w_mlp2.T.reshape(2, 16, P, 8, P).transpose(2, 0, 3, 1, 4))
    bm1_h = np.ascontiguousarray(b_mlp1.reshape(c.DHC, P).T)
    bm2_h = np.ascontiguousarray(b_mlp2.reshape(c.CH, P).T)

    HH = c.HH
    selb_h = np.zeros((2 * HH, HH, P), np.float32)
    selb2_h = np.zeros((2 * HH, HH, 64), np.float32)
    for hp in range(HH):
        selb_h[2 * hp, hp, :64] = 1.0
        selb_h[2 * hp + 1, hp, 64:] = 1.0
        selb2_h[2 * hp + 1, hp, :] = 1.0
    selb_h = bfc(selb_h)
    selb2_h = bfc(selb2_h)
    ident_h = bfc(np.eye(P, dtype=np.float32))
    identd_h = f8a(np.stack([np.eye(P, dtype=np.float32)] * 2, 0)
               .transpose(1, 0, 2))
    sh32_h = np.zeros((P, P), np.float32)
    for i in range(P):
        sh32_h[i ^ 32, i] = 1.0
    sh32_h = bfc(sh32_h)

    # rowsums for the folded memory layernorm
    wsum = wkv_eff.sum(1).astype(f32)          # [2D]
    swk_h = np.ascontiguousarray(wsum[:D].reshape(c.HH, P).T)
    wsumv_h = np.ascontiguousarray(wsum[D:][None, :])

    pos = np.arange(N, dtype=f32)
    inv = (10000.0 ** (-np.arange(0, HD, 2, dtype=f32) / HD)).astype(f32)
    freqs = pos[:, None] * inv[None, :]
    cos64 = np.concatenate([np.cos(freqs), np.cos(freqs)], 1)
    s_sgn = np.concatenate([-np.sin(freqs), np.sin(freqs)], 1)
    c_pair = np.ascontiguousarray(np.tile(cos64.T, (2, 1)).astype(f32))
    s_pair = np.ascontiguousarray(np.tile(s_sgn.T, (2, 1)).astype(f32))
    scale = f32(1.0 / np.sqrt(HD))

    in_maps = []
    for b in range(c.B):
        xb = q_x[b]
        mu_x = xb.mean(-1).astype(f32)
        rs_x = (1.0 / np.sqrt(xb.var(-1) + c.eps)).astype(f32)
        mem = np.concatenate([h_content[b], h_obs[b]], 0)
        mu_m = mem.mean(-1).astype(f32)
        rs_m = (1.0 / np.sqrt(mem.var(-1) + c.eps)).astype(f32)
        mrs_m = (mu_m * rs_m).astype(f32)
        rs_cols_h = np.ascontiguousarray((rs_m / WS).reshape(2 * c.KK, P).T)
        mrs_cols_h = np.ascontiguousarray(mrs_m.reshape(2 * c.KK, P).T)
        mTQQ = np.maximum(M_QQ[b].T, -100.0).astype(f32)
        mThyb = np.maximum(M_hyb[b].T, -100.0).astype(f32)

        hc_s_h = f8a(
            h_content[b].T.reshape(8, P, 2, 512).transpose(1, 2, 0, 3)
        )
        ho_s_h = f8a(h_obs[b].T.reshape(8, P, 2, 512).transpose(1, 2, 0, 3))
        for s in range(2):
            own = np.arange(s * SQ, (s + 1) * SQ)
            rest = np.concatenate(
                [np.arange(0, s * SQ), np.arange((s + 1) * SQ, N)]
            )
            perm = np.concatenate([own, rest]).astype(np.int64)

            def swm(m):  # [N, SQ] mask -> [P, 8, 2, SQ] fp8, slot-1 zero
                t = m.reshape(8, P, SQ).transpose(1, 0, 2)
                z = np.zeros((P, 8, 2, SQ), np.float32)
                z[:, :, 0, :] = t
                return f8a(z)

            xhb = ((xb - mu_x[:, None]) * rs_x[:, None]).astype(f32)
            im = {
                "x_s": bfc(
                    xb.T[:, perm].reshape(8, P, N).transpose(1, 0, 2)
                ),
                "xh_s": bfc(
                    xhb.T[:, perm].reshape(8, P, N).transpose(1, 0, 2)
                ),
                "tc_s": f8a(
                    t_cond[b].T[:, perm].reshape(2, P, N).transpose(1, 0, 2)
                ),
                "hc_s": hc_s_h, "ho_s": ho_s_h,
                "wada_s": wada_s_h, "wqkv_qk": wqkv_qk_h,
                "wqkv_v": wqkv_v_h, "wself_o": wself_o_h,
                "wq_c": wq_c_h, "wkv_k": wkv_k_h, "wkv_v": wkv_v_h,
                "wcross_o": wcross_o_h, "wm1_s": wm1_s_h, "wm2_s": wm2_s_h,
                "bada": bada_h, "bada_u": bada_u_h,
                "bm1": bm1_h, "bm2": bm2_h,
                "cqt": bfc(c_pair[:, perm[:SQ]] * scale),
                "sqt": bfc(s_pair[:, perm[:SQ]] * scale),
                "ckts": bfc(c_pair[:, perm]),
                "skts": bfc(s_pair[:, perm]),
                "cktm": bfc(c_pair), "sktm": bfc(s_pair),
                "mself_s": swm(mTQQ[perm][:, perm[:SQ]]),
                "mhc_s": swm(mThyb[:N][:, perm[:SQ]]),
                "mho_s": swm(mThyb[N:][:, perm[:SQ]]),
                "la_self": bfc(rs_x[perm][None, :]),
                "lb_self": bfc(mu_x[perm][None, :]),
                "la_mc": bfc((rs_m / WS)[:N][None, :]),
                "lb_mc": bfc(mrs_m[:N][None, :]),
                "la_mo": bfc((rs_m / WS)[N:][None, :]),
                "lb_mo": bfc(mrs_m[N:][None, :]),
                "swk": swk_h, "wsumv": wsumv_h,
                "rs_cols": rs_cols_h, "mrs_cols": mrs_cols_h,
                "selb": selb_h, "selb2": selb2_h,
                "ident": ident_h, "sh32": sh32_h, "identd": identd_h,
            }
            in_maps.append(im)
    return in_maps


_PROGRAM_CACHE = {}


def get_program(cfg: Cfg):
    key = (cfg.N, cfg.D, cfg.H)
    if key not in _PROGRAM_CACHE:
        _PROGRAM_CACHE[key] = build_program(cfg)
    return _PROGRAM_CACHE[key]


def assemble(cfg: Cfg, results):
    c = cfg
    out = np.zeros((c.B, c.N, c.D), np.float32)
    for b in range(c.B):
        for s in range(2):
            o = results[2 * b + s]["out"]
            out[b, s * c.SQ : (s + 1) * c.SQ, :] = o.T
    return out


def kernel(**inputs) -> np.ndarray:
    cfg = Cfg(mini=False)
    nc = get_program(cfg)
    in_maps = host_prep(cfg, inputs)
    res = bass_utils.run_bass_kernel_spmd(
        nc, in_maps, core_ids=list(range(cfg.n_cores)), trace=False
    )
    return assemble(cfg, res.results)



# revision 42
# speedup vs baseline: 1.1784x; 1.1784x over previous
"""Trainium2 Bass kernel for a DiT-style transformer block (adaLN modulation,
RoPE self-attention with additive rank mask, hybrid cross-attention to
[clean|observed] memory, gated MLP).

Sharding: 8 cores = 4 batches x 2 sequence-halves. Each core computes the
block output for its 512 query tokens of one batch. Per-core token order is
permuted (host side) so the core's own tokens come first, which keeps the
program SPMD-static across cores.

Layout: activations live feature-major ("T-layout", [feature, token]) so all
matmuls contract along partitions; weights are host-pre-swizzled to
[128, blocks, free] so each load is one contiguous DMA per partition.

Precision: every weight GEMM (ada, qkv, kv, out-projs, gates, mlp) runs as
fp8e4 DoubleRow (two k-chunks per pass via [P, 2, free] APs, ~2x bf16 PE
rate). fp8 weights are host-scaled by WS=64 (e4m3 min-normal is 2^-6); the
1/WS unscale folds into existing ACT/DVE eviction ops or host-side rstd
tables. Scores stay bf16 (64-deep contraction can't pack); p@v packs key-
chunk pairs in fp8 DR. Rel err ~1.5e-2 (gate 2e-2).

Attention: the additive mask is injected into the scores PSUM with an
[I, I] x [mask, 0] fp8-DR matmul before exp, so softmax needs no
elementwise mask op. exp(s - ln16) keeps fp8 probs < 240; the 1/16 cancels
in normalization. Denominators come free from a ones-column appended to
each head's value block (p@v row 64); normalization is batched via one-hot
PE broadcasts. RoPE's xor-32 shift is a PE permutation matmul (no DMAs).

Scheduling: the full cross-attention K/V (memory-LN folded:
kv = rs_t*(W@mem) - (mu*rs)_t*rowsum(W)) is built during phase 1 so its
DVE-heavy rope/fold work overlaps phase-1 attention (ACT/PE-bound); cross
attention then runs head-pair-major accumulating all 16 key chunks in PSUM.
DVE elementwise ops keep all operands bf16-in-SBUF where possible (2x DVE
rate); phase-1 LN is host-precomputed (xh_s). Residual stream stays in SBUF
across all three phases. ~0.77ms from the 1.13ms bf16 baseline.
"""

import numpy as np
import ml_dtypes
from contextlib import ExitStack

BF = ml_dtypes.bfloat16

from concourse import bacc, mybir
import concourse.bass as bass
import concourse.tile as tile
from concourse import bass_utils

F32 = mybir.dt.float32
F32R = mybir.dt.float32r
BF16 = mybir.dt.bfloat16
F8 = mybir.dt.float8e4
F8H = ml_dtypes.float8_e4m3
AF = mybir.ActivationFunctionType
OP = mybir.AluOpType
DR = mybir.MatmulPerfMode.DoubleRow

P = 128
WS = 64.0    # fp8 weight pre-scale (host multiplies weights by WS)
WSI = 1.0 / WS


class Cfg:
    def __init__(self, mini=False):
        if mini:
            self.B, self.N, self.D, self.H, self.HD = 2, 256, 256, 4, 64
            self.COND = 128
        else:
            self.B, self.N, self.D, self.H, self.HD = 4, 1024, 1024, 16, 64
            self.COND = 256
        self.DH = 4 * self.D
        self.SQ = self.N // 2            # own query tokens per core
        self.CH = self.D // P            # d-chunks
        self.HH = self.H * self.HD // P  # head-pair chunks (= H // 2)
        self.KK = self.N // P            # key chunks per N tokens
        self.NF = self.N // self.SQ      # token-free blocks of SQ (=2)
        self.CC = self.COND // P
        self.DHC = self.DH // P
        self.QKK = self.SQ // P          # key chunks per memory quarter
        self.n_cores = 2 * self.B
        self.eps = 1e-5


def _dma_bcast(nc, out_tile, dram_ap, off, n):
    """DMA dram row [1, off:off+n] broadcast to all partitions [P, n]."""
    src = bass.AP(
        tensor=dram_ap.tensor, offset=dram_ap.offset + off, ap=[[0, P], [1, n]]
    )
    nc.gpsimd.dma_start(out=out_tile, in_=src)


def _shift32_dma(nc, dst, src):
    """dst[p] = src[p xor-32 within each 64-block], [128, F] SBUF tiles."""
    for blk in range(2):
        b = blk * 64
        nc.sync.dma_start(out=dst[b : b + 32, :], in_=src[b + 32 : b + 64, :])
        nc.sync.dma_start(out=dst[b + 32 : b + 64, :], in_=src[b : b + 32, :])


def r(ap):
    """fp32 view of an f32r AP for DVE/ACT/gpsimd input reads."""
    return ap.bitcast(F32)


def build_program(cfg: Cfg):
    c = cfg
    nc = bacc.Bacc(
        "TRN2",
        target_bir_lowering=False,
        debug=False,
        enable_asserts=True,
        num_devices=c.n_cores,
    )

    def din(name, shape, dt=F32R):
        return nc.dram_tensor(name, shape, dt, kind="ExternalInput").ap()

    # pre-swizzled (host) bf16 weights/activations: [P, blocks..., free] so
    # every DMA is one contiguous run per partition.
    x_s = din("x_s", [P, c.CH, c.N], BF16)
    xh_s = din("xh_s", [P, c.CH, c.N], BF16)  # (x-mu)*rstd, host-side
    tc_s = din("tc_s", [P, c.CC, c.N], F8)
    hc_s = din("hc_s", [P, c.NF, c.CH, c.SQ], F8)
    ho_s = din("ho_s", [P, c.NF, c.CH, c.SQ], F8)
    wada_s = din("wada_s", [P, 9, c.CH, c.CC, P], F8)
    wqkv_qk = din("wqkv_qk", [P, 2, c.HH, c.CH, P], F8)
    wqkv_v = din("wqkv_v", [P, 2, 4, 2, 512], F8)
    wself_o = din("wself_o", [P, c.CH, c.HH, P], F8)
    wq_c = din("wq_c", [P, c.HH, c.CH, P], F8)
    wkv_k = din("wkv_k", [P, c.HH, c.CH, P], F8)
    wkv_v = din("wkv_v", [P, 2, 4, 2, 512], F8)
    wcross_o = din("wcross_o", [P, c.CH, c.HH, P], F8)
    wm1_s = din("wm1_s", [P, c.DHC, c.CH, P], F8)
    wm2_s = din("wm2_s", [P, 2, c.CH, 16, P], F8)
    mself_s = din("mself_s", [P, c.KK, 2, c.SQ], F8)
    mhc_s = din("mhc_s", [P, c.KK, 2, c.SQ], F8)
    mho_s = din("mho_s", [P, c.KK, 2, c.SQ], F8)
    bada = din("bada", [P, 9 * c.CH], F32)     # pre-scaled by WS
    bada_u = din("bada_u", [P, 9 * c.CH], F32)  # unscaled
    bm1 = din("bm1", [P, c.DHC], F32)
    bm2 = din("bm2", [P, c.CH], F32)
    cqt = din("cqt", [P, c.SQ], BF16)
    sqt = din("sqt", [P, c.SQ], BF16)
    ckts = din("ckts", [P, c.N], BF16)
    skts = din("skts", [P, c.N], BF16)
    cktm = din("cktm", [P, c.N], BF16)
    sktm = din("sktm", [P, c.N], BF16)
    la_self = din("la_self", [1, c.N], BF16)  # rstd per own-order token
    lb_self = din("lb_self", [1, c.N], BF16)  # mean per own-order token
    la_mc = din("la_mc", [1, c.N], BF16)      # rstd, clean memory
    lb_mc = din("lb_mc", [1, c.N], BF16)      # mean*rstd, clean memory
    la_mo = din("la_mo", [1, c.N], BF16)
    lb_mo = din("lb_mo", [1, c.N], BF16)
    swk = din("swk", [P, c.HH], F32)          # rowsum(Wk) per k-feature
    wsumv = din("wsumv", [1, c.H * c.HD], F32)  # rowsum(Wv) per v-feature
    rs_cols = din("rs_cols", [P, 2 * c.KK], F32)    # mem rstd, column layout
    mrs_cols = din("mrs_cols", [P, 2 * c.KK], F32)  # mem mean*rstd, columns
    selb = din("selb", [2 * c.HH, c.HH, P], BF16)   # one-hot pair selector
    selb2 = din("selb2", [2 * c.HH, c.HH, 64], BF16)  # odd-head selector
    ident = din("ident", [P, P], BF16)              # identity (mask inject)
    identd = din("identd", [P, 2, P], F8)           # identity pair (DR inject)
    sh32 = din("sh32", [P, P], BF16)                # xor-32 partition permute
    out_d = nc.dram_tensor("out", [c.D, c.SQ], F32, kind="ExternalOutput").ap()

    with ExitStack() as ctx:
        tc = ctx.enter_context(tile.TileContext(nc))
        persist = ctx.enter_context(tc.tile_pool(name="persist", bufs=1))
        ws = ctx.enter_context(tc.tile_pool(name="wstream", bufs=1))
        tw_pool = ctx.enter_context(tc.tile_pool(name="tw", bufs=5))
        rsp = ctx.enter_context(tc.tile_pool(name="rsp", bufs=1))
        xkv = ctx.enter_context(tc.tile_pool(name="xkv", bufs=1))
        small = ctx.enter_context(tc.tile_pool(name="small", bufs=1))

        def wtile():
            return ws.tile([P, P], BF16, name="wt", tag="wt", bufs=8)

        def wbtile(nk):
            return ws.tile([P, nk, P], BF16, name=f"wb{nk}", tag=f"wb{nk}",
                           bufs=3)


        def tw():
            return tw_pool.tile([P, c.SQ], F32, name="tw", tag="tw")

        def twb():
            return tw_pool.tile([P, c.SQ], BF16, name="twb", tag="twb",
                                bufs=8)

        # ---------- persistent preloads ----------
        TC = persist.tile([P, c.CC, c.N], F8)
        nc.sync.dma_start(out=TC, in_=tc_s)
        CQ = persist.tile([P, c.SQ], BF16)
        nc.sync.dma_start(out=CQ, in_=cqt)
        SQt = persist.tile([P, c.SQ], BF16)
        nc.sync.dma_start(out=SQt, in_=sqt)
        BADA = persist.tile([P, 9 * c.CH], F32)
        nc.sync.dma_start(out=BADA, in_=bada)
        BADAU = persist.tile([P, 9 * c.CH], F32)
        nc.sync.dma_start(out=BADAU, in_=bada_u)
        BM1 = persist.tile([P, c.DHC], F32)
        nc.sync.dma_start(out=BM1, in_=bm1)
        BM2 = persist.tile([P, c.CH], F32)
        nc.sync.dma_start(out=BM2, in_=bm2)

        # SBUF-resident residual stream (x after self-attn / after cross)
        XC1 = persist.tile([P, c.CH, c.SQ], BF16)
        XC2 = persist.tile([P, c.CH, c.SQ], BF16)

        EPS = persist.tile([P, 1], F32)
        nc.vector.memset(EPS, 1e-5)
        ones_f32 = persist.tile([P, 16], F32)
        nc.vector.memset(ones_f32, 1.0)
        ONEB = persist.tile([P, 1], BF16)
        nc.vector.tensor_copy(ONEB, ones_f32[:, 0:1])
        ONES16 = persist.tile([P, 16], BF16)
        nc.vector.tensor_copy(ONES16, ones_f32)
        EXPB = persist.tile([P, 1], F32)
        nc.vector.memset(EXPB, -2.7725887)
        SELB = persist.tile([2 * c.HH, c.HH, P], BF16)
        nc.sync.dma_start(out=SELB, in_=selb)
        SELB2 = persist.tile([2 * c.HH, c.HH, 64], BF16)
        nc.sync.dma_start(out=SELB2, in_=selb2)
        IDENT = persist.tile([P, P], BF16)
        nc.sync.dma_start(out=IDENT, in_=ident)
        IDENTD = persist.tile([P, 2, P], F8)
        nc.sync.dma_start(out=IDENTD, in_=identd)
        SH32 = persist.tile([P, P], BF16)
        nc.sync.dma_start(out=SH32, in_=sh32)

        # ---------- helpers ----------
        def ada_modulate(q_sh, q_sc, x_src, x_nf, la_b, lb_b, xn_dst,
                         xh_src=None):
            """xn = x*sc1 - m*sc1 + sh, with sc1 = rs*w*(1+sc).

            la_b(cols) -> [P, SQ] rstd broadcast AP; lb_b(cols) -> mean.
            x_src(j, tf) / xn_dst(j, tf): [P, SQ] APs.
            """
            with tc.tile_pool(name="ps_ada", bufs=1, space="PSUM") as psa:
                wsh = ws.tile([P, c.CH, c.CC, P], F8, name="wadb",
                              tag="wadb", bufs=2)
                nc.sync.dma_start(out=wsh, in_=wada_s[:, q_sh])
                wsc = ws.tile([P, c.CH, c.CC, P], F8, name="wadb",
                              tag="wadb", bufs=2)
                nc.sync.dma_start(out=wsc, in_=wada_s[:, q_sc])
                for j in range(c.CH):
                    ps_sh = [
                        psa.tile([P, c.SQ], F32, name=f"ps_sh{t}", tag=f"ps_sh{t}")
                        for t in range(x_nf)
                    ]
                    ps_sc = [
                        psa.tile([P, c.SQ], F32, name=f"ps_sc{t}", tag=f"ps_sc{t}")
                        for t in range(x_nf)
                    ]
                    for tf in range(x_nf):
                        nc.tensor.matmul(
                            ps_sh[tf], wsh[:, j],
                            TC[:, :, tf * c.SQ : (tf + 1) * c.SQ],
                            start=True, stop=True, perf_mode=DR,
                        )
                    for tf in range(x_nf):
                        nc.tensor.matmul(
                            ps_sc[tf], wsc[:, j],
                            TC[:, :, tf * c.SQ : (tf + 1) * c.SQ],
                            start=True, stop=True, perf_mode=DR,
                        )
                    for tf in range(x_nf):
                        cols = slice(tf * c.SQ, (tf + 1) * c.SQ)
                        # xn = (x - m)*(yc*la) + sh0. Both psum evictions
                        # ride ACT (idle here); DVE ops stay all-bf16 SBUF
                        # so they hit the 2x DVE rate; xd runs on GpSimd.
                        yc = twb()
                        nc.scalar.activation(
                            yc, ps_sc[tf], AF.Identity, scale=WSI,
                            bias=BADAU[:, q_sc * c.CH + j : q_sc * c.CH + j + 1],
                        )
                        sh0 = twb()
                        nc.scalar.activation(
                            sh0, ps_sh[tf], AF.Identity, scale=WSI,
                            bias=BADAU[:, q_sh * c.CH + j : q_sh * c.CH + j + 1],
                        )
                        if xh_src is not None:
                            t = twb()
                            nc.vector.tensor_mul(t, xh_src(j, tf), yc)
                        else:
                            xd = twb()
                            nc.vector.tensor_sub(
                                xd, x_src(j, tf), lb_b(cols)
                            )
                            sc1 = twb()
                            nc.vector.tensor_mul(sc1, yc, la_b(cols))
                            t = twb()
                            nc.vector.tensor_mul(t, xd, sc1)
                        nc.vector.tensor_add(xn_dst(j, tf), t, sh0)

        def gate_wtile(q_g):
            wg = ws.tile([P, c.CH, c.CC, P], F8, name="wadb", tag="wadb",
                         bufs=2)
            nc.sync.dma_start(out=wg, in_=wada_s[:, q_g])
            return wg

        def ada_gate_one(q_g, wg, j, psg, unscale):
            """Return a [P, SQ] f32 tile holding gate chunk j on demand.

            unscale folds both the gate's own fp8 weight scale and the
            downstream psum's weight scale into the gate values."""
            ps = psg.tile([P, c.SQ], F32, name="ps_g", tag="ps_g")
            nc.tensor.matmul(
                ps, wg[:, j], TC[:, :, 0 : c.SQ],
                start=True, stop=True, perf_mode=DR,
            )
            g = tw()
            nc.vector.tensor_scalar(
                g, ps, BADA[:, q_g * c.CH + j : q_g * c.CH + j + 1],
                unscale, op0=OP.add, op1=OP.mult,
            )
            return g

        def rope_evict(zsrc, hh, cols_t, ctab, stab, dst, psp):
            """dst[:, hh, cols_t] = zsrc*cos + shift32(zsrc)*sin_signed.

            The xor-32 partition shift runs on the PE (permutation matmul)
            instead of 4 small SBUF-to-SBUF DMAs."""
            ps2 = psp.tile([P, c.SQ], F32, name="ps_sh32", tag="ps_sh32",
                           bufs=2)
            nc.tensor.matmul(ps2, SH32, zsrc, start=True, stop=True)
            t1 = twb()
            nc.vector.tensor_mul(t1, zsrc, ctab)
            t2 = twb()
            nc.vector.tensor_mul(t2, ps2, stab)
            nc.vector.tensor_add(dst[:, hh, cols_t], t1, t2)

        def proj_rope(wsel, n_free, ctab, stab, dst, src_tile):
            """dst[:, hh, :] = rope(W[:, cols].T @ src), head-pair chunks."""
            nf = n_free // c.SQ
            with tc.tile_pool(name="ps_qk", bufs=3, space="PSUM") as psq:
                wt = ws.tile([P, c.HH, c.CH, P], F8, name="wpr", tag="wpr",
                             bufs=1)
                nc.sync.dma_start(out=wt, in_=wsel)
                for hh in range(c.HH):
                    for tf in range(nf):
                        ps = psq.tile([P, c.SQ], F32, name="ps_qk", tag="ps_qk")
                        for kp in range(c.CH // 2):
                            nc.tensor.matmul(
                                ps, wt[:, hh, 2 * kp : 2 * kp + 2, :],
                                src_tile[:, 2 * kp : 2 * kp + 2,
                                         tf * c.SQ : (tf + 1) * c.SQ],
                                start=(kp == 0), stop=(kp == c.CH // 2 - 1),
                                perf_mode=DR,
                            )
                        cols = slice(tf * c.SQ, (tf + 1) * c.SQ)
                        traw = twb()
                        nc.scalar.activation(traw, ps, AF.Copy, scale=WSI)
                        rope_evict(
                            traw, hh, cols, ctab[:, cols], stab[:, cols],
                            dst, psq,
                        )

        def vproj_self(src_tile, vdst, wvp):
            """Token-major value projection from resident XN; ones cols."""
            ntt = c.KK
            ffw = min(512, c.H * c.HD)
            nff = (c.H * c.HD) // ffw
            hpf = ffw // 64
            for tt in range(ntt):
                ap = vdst[:, tt, :].rearrange("p (h e) -> p h e", e=65)[:, :, 64:65]
                nc.vector.tensor_copy(ap, ONES16[:, 0 : c.H])
            wt_all = wvp.tile([P, 2, 4, 2, ffw], F8, name="wv", tag="wv",
                              bufs=1)
            nc.sync.dma_start(out=wt_all, in_=wqkv_v)
            with tc.tile_pool(name="ps_v", bufs=8, space="PSUM") as psv:
                for ff in range(nff):
                    pss = [
                        psv.tile([P, ffw], F32, name="ps_v", tag="ps_v")
                        for _ in range(ntt)
                    ]
                    for kp in range(c.CH // 2):
                        for tt in range(ntt):
                            nc.tensor.matmul(
                                pss[tt],
                                src_tile[:, 2 * kp : 2 * kp + 2,
                                         tt * P : (tt + 1) * P],
                                wt_all[:, ff, kp, :, :],
                                start=(kp == 0), stop=(kp == c.CH // 2 - 1),
                                perf_mode=DR,
                            )
                    for tt in range(ntt):
                        ap = (
                            vdst[:, tt, ff * hpf * 65 : (ff + 1) * hpf * 65]
                            .rearrange("p (h e) -> p h e", e=65)[:, :, 0:64]
                        )
                        nc.vector.tensor_scalar_mul(ap, pss[tt], WSI)

        def attention_hp(hp, khat, vtile, qhat, masks, n_kk, ps_o1, ps_o2,
                         tp_pool, first, last, pss_bufs=3):
            """One head pair, software-pipelined: p@v (fp8 DoubleRow over
            key-chunk pairs) lags scores by one pair so the PE has
            independent work while ACT runs exp on the current pair."""
            h1, h2 = 2 * hp, 2 * hp + 1
            npair = n_kk // 2

            def pv(pc, pt):
                nc.tensor.matmul(
                    ps_o1,
                    vtile[:, 2 * pc : 2 * pc + 2, h1 * 65 : (h1 + 1) * 65],
                    pt[:, :, 0 : c.SQ],
                    start=(first and pc == 0),
                    stop=(last and pc == npair - 1),
                    perf_mode=DR,
                )
                nc.tensor.matmul(
                    ps_o2,
                    vtile[:, 2 * pc : 2 * pc + 2, h2 * 65 : (h2 + 1) * 65],
                    pt[:, :, c.SQ : 2 * c.SQ],
                    start=(first and pc == 0),
                    stop=(last and pc == npair - 1),
                    perf_mode=DR,
                )

            with tc.tile_pool(name="ps_s", bufs=pss_bufs,
                              space="PSUM") as pss:
                prev = None
                for pc in range(npair):
                    pt = tp_pool.tile(
                        [P, 2, 2 * c.SQ], F8, name="t_p", tag="t_p", bufs=2
                    )
                    for j in range(2):
                        kkc = 2 * pc + j
                        ps = pss.tile([P, 2 * c.SQ], F32, name="ps_s",
                                      tag="ps_s")
                        ks = slice(kkc * P, (kkc + 1) * P)
                        # additive mask injected into PSUM via identity
                        # matmul, then scores accumulate on top;
                        # exp(scores+mask) needs no mask multiply after.
                        nc.tensor.matmul(
                            ps[:, 0 : c.SQ], IDENTD, masks[:, kkc],
                            start=True, stop=False, perf_mode=DR,
                        )
                        nc.tensor.matmul(
                            ps[:, c.SQ : 2 * c.SQ], IDENTD, masks[:, kkc],
                            start=True, stop=False, perf_mode=DR,
                        )
                        nc.tensor.matmul(
                            ps[:, 0 : c.SQ],
                            khat[0:64, hp, ks], qhat[0:64, hp, :],
                            start=False, stop=True,
                        )
                        nc.tensor.matmul(
                            ps[:, c.SQ : 2 * c.SQ],
                            khat[64:128, hp, ks], qhat[64:128, hp, :],
                            start=False, stop=True,
                        )
                        # exp(s - ln 16): keeps fp8 probs < 240; the 1/16
                        # cancels in the softmax normalization.
                        nc.scalar.activation(pt[:, j, :], ps, AF.Exp,
                                             bias=EXPB[:, 0:1])
                    if prev is not None:
                        pv(*prev)
                    prev = (pc, pt)
                pv(*prev)

        def evict_unnorm(ps_o, hp, second, odst, den, tp_pool):
            """Stage unnormalized o rows into odst and the denominator row
            into den[2hp+second]. Normalization happens batched later."""
            h = 2 * hp + (1 if second else 0)
            dstage = tp_pool.tile(
                [65, c.SQ], F32, name="t_dstage", tag="t_dstage", bufs=2
            )
            nc.vector.tensor_copy(dstage[64:65, :], ps_o[64:65, :])
            nc.sync.dma_start(out=den[h : h + 1, :], in_=dstage[64:65, :])
            if not second:
                nc.vector.tensor_copy(odst[0:64, hp, :], ps_o[0:64, :])
            else:
                st = tp_pool.tile(
                    [64, c.SQ], BF16, name="t_onorm", tag="t_onorm", bufs=2
                )
                nc.vector.tensor_copy(st, ps_o[0:64, :])
                nc.sync.dma_start(out=odst[64:128, hp, :], in_=st)

        def normalize_batch(osrc_u, odst, den, deni, tp_pool, n_hp):
            """odst[:, hp, :] = osrc_u * 1/den rows (PE one-hot broadcast)."""
            with nc.allow_low_precision(reason="bf16 softmax denominators"):
                nc.vector.reciprocal(deni, den)
            with tc.tile_pool(name="ps_nb", bufs=2, space="PSUM") as pnb:
                for hp in range(n_hp):
                    ps_rb = pnb.tile([P, c.SQ], F32, name="ps_rb",
                                     tag="ps_rb")
                    nc.tensor.matmul(
                        ps_rb, SELB[:, hp, :], deni, start=True, stop=True
                    )
                    nc.vector.tensor_mul(
                        odst[:, hp, :], osrc_u[:, hp, :], ps_rb
                    )

        def out_proj_residual(wo_s, osrc, g_src, xr, xdst, stats=None):
            with tc.tile_pool(name="ps_op", bufs=3, space="PSUM") as pso:
                wt = ws.tile([P, c.CH, c.HH, P], F8, name="wop", tag="wop",
                             bufs=1)
                nc.sync.dma_start(out=wt, in_=wo_s)
                for j in range(c.CH):
                    ps = pso.tile([P, c.SQ], F32, name="ps_op", tag="ps_op")
                    for hp in range(c.HH // 2):
                        nc.tensor.matmul(
                            ps, wt[:, j, 2 * hp : 2 * hp + 2, :],
                            osrc[:, 2 * hp : 2 * hp + 2, :],
                            start=(hp == 0), stop=(hp == c.HH // 2 - 1),
                            perf_mode=DR,
                        )
                    t = tw()
                    nc.vector.tensor_mul(t, ps, g_src(j))
                    nc.vector.tensor_add(xdst(j), t, xr(j))

        def ln_stats_make(psst, stp):
            return {
                "ps1": psst.tile([1, c.SQ], F32, name="ps_st1", tag="ps_st1"),
                "ps2": psst.tile([1, c.SQ], F32, name="ps_st2", tag="ps_st2"),
                "stp": stp,
            }

        def ln_stats_accum(st, j, xa):
            sq = st["stp"].tile([P, c.SQ], BF16, name="t_sq", tag="t_sq",
                                bufs=2)
            nc.vector.tensor_mul(sq, xa, xa)
            nc.tensor.matmul(
                st["ps1"], ONEB, xa, start=(j == 0), stop=(j == c.CH - 1)
            )
            nc.tensor.matmul(
                st["ps2"], ONEB, sq, start=(j == 0), stop=(j == c.CH - 1)
            )

        def ln_stats_finish(st):
            """[P, SQ] broadcast (rstd, mean) bf16 tiles from the psums."""
            rs_b = rsp.tile([P, c.SQ], BF16, name="t_rsb", tag="t_rsb")
            m_b = rsp.tile([P, c.SQ], BF16, name="t_mb", tag="t_mb")
            stp = st["stp"]
            m = stp.tile([1, c.SQ], F32, name="s_m", tag="s_m")
            nc.vector.tensor_scalar_mul(m, st["ps1"][0:1, :], 1.0 / c.D)
            e2 = stp.tile([1, c.SQ], F32, name="s_a", tag="s_a")
            nc.vector.tensor_scalar_mul(e2, st["ps2"][0:1, :], 1.0 / c.D)
            msq = stp.tile([1, c.SQ], F32, name="s_b", tag="s_b")
            nc.vector.tensor_mul(msq, m, m)
            var = stp.tile([1, c.SQ], F32, name="s_c", tag="s_c")
            nc.vector.tensor_sub(var, e2, msq)
            sd = stp.tile([1, c.SQ], F32, name="s_d", tag="s_d")
            nc.scalar.activation(sd, var, AF.Sqrt, bias=EPS[0:1, :])
            rs = stp.tile([1, c.SQ], F32, name="s_e", tag="s_e")
            nc.vector.reciprocal(rs, sd)
            rs16 = stp.tile([1, c.SQ], BF16, name="s_e16", tag="s_e16")
            nc.vector.tensor_copy(rs16, rs)
            m16 = stp.tile([1, c.SQ], BF16, name="s_m16", tag="s_m16")
            nc.vector.tensor_copy(m16, m)
            nc.gpsimd.partition_broadcast(rs_b, rs16, channels=P)
            nc.gpsimd.partition_broadcast(m_b, m16, channels=P)
            return rs_b, m_b

        def stream_x(dram, j, cols):
            t = tw()
            nc.sync.dma_start(out=t, in_=r(dram[j * P : (j + 1) * P, cols]))
            return t

        def stream_xr(dram, j):
            t = tw_pool.tile([P, c.SQ], F32R, name="twr", tag="twr", bufs=2)
            nc.sync.dma_start(out=t, in_=dram[j * P : (j + 1) * P, :])
            return t

        KCF = xkv.tile([P, c.HH, 2 * c.N], BF16)
        VCF = xkv.tile([P, 2 * c.KK, c.H * 65], F8)

        def build_cross_kv():
            """Build the full cross-attention K (rope'd, bf16) and V (fp8)
            from the memory streams; independent of phase 1, emitted so its
            DVE work overlaps phase-1 attention."""
            ffw = min(512, c.H * c.HD)
            nff = (c.H * c.HD) // ffw
            hpf = ffw // 64
            with tc.tile_pool(name="p2h", bufs=1) as p2h, \
                 tc.tile_pool(name="mstr", bufs=1) as mstr:
              SWK = p2h.tile([P, c.HH], F32)
              nc.sync.dma_start(out=SWK, in_=swk)
              WSVb = p2h.tile([P, c.H * c.HD], F32)
              _dma_bcast(nc, WSVb, wsumv, 0, c.H * c.HD)
              RSC = p2h.tile([P, 2 * c.KK], F32)
              nc.sync.dma_start(out=RSC, in_=rs_cols)
              MRSC = p2h.tile([P, 2 * c.KK], F32)
              nc.sync.dma_start(out=MRSC, in_=mrs_cols)
              WKV_K = p2h.tile([P, c.HH, c.CH, P], F8)
              nc.sync.dma_start(out=WKV_K, in_=wkv_k)
              WKV_V = p2h.tile([P, 2, 4, 2, 512], F8)
              nc.sync.dma_start(out=WKV_V, in_=wkv_v)
              for qq in range(2 * c.NF):
                half = qq // c.NF            # 0: clean, 1: observed
                hq = qq % c.NF               # quarter index within half
                mem_s = hc_s if half == 0 else ho_s
                la_m = la_mc if half == 0 else la_mo
                lb_m = lb_mc if half == 0 else lb_mo
                tok0 = hq * c.SQ
                ctok = slice(tok0, tok0 + c.SQ)

                MEMQ = p2h.tile([P, c.CH, c.SQ], F8, name="MEMQ",
                                tag="MEMQ", bufs=2)
                nc.sync.dma_start(out=MEMQ, in_=mem_s[:, hq])
                CKm_t = p2h.tile([P, c.SQ], BF16, name="CKm", tag="CKm",
                                 bufs=2)
                nc.sync.dma_start(out=CKm_t, in_=cktm[:, ctok])
                SKm_t = p2h.tile([P, c.SQ], BF16, name="SKm", tag="SKm",
                                 bufs=2)
                nc.sync.dma_start(out=SKm_t, in_=sktm[:, ctok])
                LAm = p2h.tile([P, c.SQ], BF16, name="LAm", tag="LAm",
                               bufs=2)
                _dma_bcast(nc, LAm, la_m, tok0, c.SQ)
                LBm = p2h.tile([P, c.SQ], BF16, name="LBm", tag="LBm",
                               bufs=2)
                _dma_bcast(nc, LBm, lb_m, tok0, c.SQ)

                with tc.tile_pool(name="ps_kp", bufs=2, space="PSUM") as pkp:
                    for hh in range(c.HH):
                        pk = pkp.tile([P, c.SQ], F32, name="ps_k",
                                      tag="ps_k")
                        for kp in range(c.CH // 2):
                            nc.tensor.matmul(
                                pk, WKV_K[:, hh, 2 * kp : 2 * kp + 2, :],
                                MEMQ[:, 2 * kp : 2 * kp + 2, :],
                                start=(kp == 0), stop=(kp == c.CH // 2 - 1),
                                perf_mode=DR,
                            )
                        # LN fold: z = ps*rs_t - (mu*rs)_t * rowsum(Wk)
                        t2 = twb()
                        nc.vector.tensor_scalar_mul(
                            t2, LBm, SWK[:, hh : hh + 1]
                        )
                        t1 = twb()
                        nc.vector.tensor_mul(t1, pk, LAm)
                        z = twb()
                        nc.vector.tensor_sub(z, t1, t2)
                        rope_evict(
                            z, hh, slice(qq * c.SQ, (qq + 1) * c.SQ),
                            CKm_t, SKm_t, KCF, pkp,
                        )

                for tt in range(c.QKK):
                    ap = VCF[:, qq * c.QKK + tt, :].rearrange(
                        "p (h e) -> p h e", e=65
                    )[:, :, 64:65]
                    nc.vector.tensor_copy(ap, ONES16[:, 0 : c.H])
                with tc.tile_pool(name="ps_v2", bufs=4, space="PSUM") as psv:
                    for ff in range(nff):
                        pss = [
                            psv.tile([P, ffw], F32, name="ps_v2",
                                     tag="ps_v2")
                            for _ in range(c.QKK)
                        ]
                        for kp in range(c.CH // 2):
                            for tt in range(c.QKK):
                                nc.tensor.matmul(
                                    pss[tt],
                                    MEMQ[:, 2 * kp : 2 * kp + 2,
                                         tt * P : (tt + 1) * P],
                                    WKV_V[:, ff, kp, :, :],
                                    start=(kp == 0),
                                    stop=(kp == c.CH // 2 - 1),
                                    perf_mode=DR,
                                )
                        for tt in range(c.QKK):
                            tok_col = half * c.KK + hq * c.QKK + tt
                            t2 = mstr.tile(
                                [P, ffw], F32, name="tvw", tag="tvw",
                                bufs=2,
                            )
                            nc.vector.tensor_scalar_mul(
                                t2, WSVb[:, ff * ffw : (ff + 1) * ffw],
                                MRSC[:, tok_col : tok_col + 1],
                            )
                            ap = VCF[
                                :, qq * c.QKK + tt,
                                ff * hpf * 65 : (ff + 1) * hpf * 65
                            ].rearrange("p (h e) -> p h e", e=65)[:, :, 0:64]
                            nc.vector.scalar_tensor_tensor(
                                out=ap, in0=pss[tt],
                                scalar=RSC[:, tok_col : tok_col + 1],
                                in1=t2, op0=OP.mult, op1=OP.subtract,
                            )


        # =======================================================
        # Phase 1: self-attention
        # =======================================================
        with tc.tile_pool(name="p1", bufs=1) as p1:
            QHAT = p1.tile([P, c.HH, c.SQ], BF16)
            KHAT = p1.tile([P, c.HH, c.N], BF16)
            VSELF = p1.tile([P, c.KK, c.H * 65], F8)
            X0 = p1.tile([P, c.CH, c.SQ], BF16)
            for jj in range(c.CH):
                nc.sync.dma_start(out=X0[:, jj], in_=x_s[:, jj, 0 : c.SQ])

            with tc.tile_pool(name="p1a", bufs=1) as p1a:
                XN = p1a.tile([P, c.CH, c.N], F8)
                XH = p1a.tile([P, c.CH, c.N], BF16)
                for jj in range(c.CH):
                    nc.sync.dma_start(out=XH[:, jj], in_=xh_s[:, jj])
                CKs_t = p1a.tile([P, c.N], BF16)
                nc.sync.dma_start(out=CKs_t, in_=ckts)
                SKs_t = p1a.tile([P, c.N], BF16)
                nc.sync.dma_start(out=SKs_t, in_=skts)
                with tc.tile_pool(name="p1ln", bufs=1) as p1ln:
                    ada_modulate(
                        0, 1,
                        None,
                        c.NF,
                        None,
                        None,
                        lambda j, tf: XN[:, j, tf * c.SQ : (tf + 1) * c.SQ],
                        xh_src=lambda j, tf: XH[
                            :, j, tf * c.SQ : (tf + 1) * c.SQ
                        ],
                    )
                proj_rope(wqkv_qk[:, 0], c.SQ, CQ, SQt, QHAT, XN)
                proj_rope(wqkv_qk[:, 1], c.N, CKs_t, SKs_t, KHAT, XN)
                with tc.tile_pool(name="wvp1", bufs=1) as wvp:
                    vproj_self(XN, VSELF, wvp)

            build_cross_kv()

            with tc.tile_pool(name="p1b", bufs=1) as p1b, \
                 tc.tile_pool(name="tp1", bufs=1) as tp1:
                MS = p1b.tile([P, c.KK, 2, c.SQ], F8)
                nc.sync.dma_start(out=MS, in_=mself_s)
                OSELFU = p1b.tile([P, c.HH, c.SQ], BF16)
                OSELF = p1b.tile([P, c.HH, c.SQ], F8)

                DENS = p1b.tile([2 * c.HH, c.SQ], F32)
                DENSI = p1b.tile([2 * c.HH, c.SQ], BF16)
                with tc.tile_pool(name="ps_oacc", bufs=2, space="PSUM") as psoa:
                    for hp in range(c.HH):
                        ps_o1 = psoa.tile(
                            [65, c.SQ], F32, name="ps_o1", tag="ps_o1"
                        )
                        ps_o2 = psoa.tile(
                            [65, c.SQ], F32, name="ps_o2", tag="ps_o2"
                        )
                        attention_hp(
                            hp, KHAT, VSELF, QHAT, MS, c.KK,
                            ps_o1, ps_o2, tp1, True, True, pss_bufs=2,
                        )
                        evict_unnorm(ps_o1, hp, False, OSELFU, DENS, tp1)
                        evict_unnorm(ps_o2, hp, True, OSELFU, DENS, tp1)
                normalize_batch(OSELFU, OSELF, DENS, DENSI, tp1, c.HH)

                with tc.tile_pool(name="ps_gx", bufs=2, space="PSUM") as psg:
                    wg1 = gate_wtile(2)
                    out_proj_residual(
                        wself_o, OSELF,
                        lambda j: ada_gate_one(2, wg1, j, psg, WSI * WSI),
                        lambda j: X0[:, j, :],
                        lambda j: XC1[:, j, :],
                    )

        # =======================================================
        # Phase 2: cross-attention (memory quarters, LN folded into proj)
        # =======================================================
        with tc.tile_pool(name="p2", bufs=1) as p2:
            with tc.tile_pool(name="ps_st", bufs=1, space="PSUM") as psst, \
                 tc.tile_pool(name="stats", bufs=1) as stp:
                st1 = ln_stats_make(psst, stp)
                for j in range(c.CH):
                    ln_stats_accum(st1, j, XC1[:, j, :])
                rs_b, m_b = ln_stats_finish(st1)
            QC = p2.tile([P, c.HH, c.SQ], BF16)
            with tc.tile_pool(name="p2q", bufs=1) as p2q:
                XNC = p2q.tile([P, c.CH, c.SQ], F8)
                ada_modulate(
                    3, 4, lambda j, tf: XC1[:, j, :], 1,
                    lambda cols: rs_b[:, cols], lambda cols: m_b[:, cols],
                    lambda j, tf: XNC[:, j, :],
                )
                proj_rope(wq_c, c.SQ, CQ, SQt, QC, XNC)

            MKF = p2.tile([P, 2 * c.KK, 2, c.SQ], F8)
            nc.sync.dma_start(out=MKF[:, 0 : c.KK], in_=mhc_s)
            nc.sync.dma_start(out=MKF[:, c.KK :], in_=mho_s)

            with tc.tile_pool(name="p2b", bufs=1) as p2b, \
                 tc.tile_pool(name="tp2", bufs=1) as tp2:
                OCU = p2b.tile([P, c.HH, c.SQ], BF16)
                OC = p2b.tile([P, c.HH, c.SQ], F8)
                DENC = p2b.tile([2 * c.HH, c.SQ], F32)
                DENCI = p2b.tile([2 * c.HH, c.SQ], BF16)
                with tc.tile_pool(name="ps_oc", bufs=2, space="PSUM") as psoc:
                    for hp in range(c.HH):
                        ps_o1 = psoc.tile(
                            [65, c.SQ], F32, name="ps_oc1", tag="ps_oc1"
                        )
                        ps_o2 = psoc.tile(
                            [65, c.SQ], F32, name="ps_oc2", tag="ps_oc2"
                        )
                        attention_hp(
                            hp, KCF, VCF, QC, MKF, 2 * c.KK,
                            ps_o1, ps_o2, tp2, True, True, pss_bufs=2,
                        )
                        evict_unnorm(ps_o1, hp, False, OCU, DENC, tp2)
                        evict_unnorm(ps_o2, hp, True, OCU, DENC, tp2)
                normalize_batch(OCU, OC, DENC, DENCI, tp2, c.HH)
                with tc.tile_pool(name="ps_gx", bufs=2, space="PSUM") as psg:
                    wg2 = gate_wtile(5)
                    out_proj_residual(
                        wcross_o, OC,
                        lambda j: ada_gate_one(5, wg2, j, psg, WSI * WSI),
                        lambda j: XC1[:, j, :],
                        lambda j: XC2[:, j, :],
                    )

        # =======================================================
        # Phase 3: MLP (two hidden halves, SBUF accumulation)
        # =======================================================
        with tc.tile_pool(name="p3", bufs=1) as p3:
            with tc.tile_pool(name="ps_st", bufs=1, space="PSUM") as psst, \
                 tc.tile_pool(name="stats", bufs=1) as stp:
                st2 = ln_stats_make(psst, stp)
                for j in range(c.CH):
                    ln_stats_accum(st2, j, XC2[:, j, :])
                rs_b, m_b = ln_stats_finish(st2)
            OUT_ACC = p3.tile([P, c.CH, c.SQ], F32)

            with tc.tile_pool(name="p3x", bufs=1) as p3x:
                XNM = p3x.tile([P, c.CH, c.SQ], F8)
                ada_modulate(
                    6, 7, lambda j, tf: XC2[:, j, :], 1,
                    lambda cols: rs_b[:, cols], lambda cols: m_b[:, cols],
                    lambda j, tf: XNM[:, j, :],
                )
                outv = out_d.rearrange("(k p) q -> p k q", p=P)
                nhalf = c.DHC // 2
                # streamed weight chunks + disjoint psum pools for m1/m2 so
                # half boundaries pipeline instead of serializing on 4MB DMAs
                with tc.tile_pool(name="p3h", bufs=1) as p3h, \
                     tc.tile_pool(name="ps_m1", bufs=3, space="PSUM") as psm, \
                     tc.tile_pool(name="ps_m2", bufs=3, space="PSUM") as psm2:
                    for half in range(2):
                        HT = p3h.tile([P, nhalf, c.SQ], F8, name="HT",
                                      tag="HT", bufs=2)
                        for jg in range(2):
                            wm1t = p3h.tile([P, 8, c.CH, P], F8,
                                            name="wm1c", tag="wm1c", bufs=2)
                            j0 = half * nhalf + jg * 8
                            nc.sync.dma_start(
                                out=wm1t, in_=wm1_s[:, j0 : j0 + 8]
                            )
                            for j8 in range(8):
                                jj = jg * 8 + j8
                                gj = half * nhalf + jj
                                ps = psm.tile(
                                    [P, c.SQ], F32, name="ps_m1", tag="ps_m1"
                                )
                                for kp in range(c.CH // 2):
                                    nc.tensor.matmul(
                                        ps, wm1t[:, j8, 2 * kp : 2 * kp + 2, :],
                                        XNM[:, 2 * kp : 2 * kp + 2, :],
                                        start=(kp == 0),
                                        stop=(kp == c.CH // 2 - 1),
                                        perf_mode=DR,
                                    )
                                nc.scalar.activation(
                                    HT[:, jj, :], ps, AF.Gelu_apprx_tanh,
                                    bias=BM1[:, gj : gj + 1], scale=WSI,
                                )
                        for jg in range(4):
                            wm2t = p3h.tile([P, 2, nhalf, P], F8,
                                            name="wm2c", tag="wm2c", bufs=2)
                            nc.sync.dma_start(
                                out=wm2t,
                                in_=wm2_s[:, half, jg * 2 : (jg + 1) * 2],
                            )
                            for j2 in range(2):
                                j = jg * 2 + j2
                                ps = psm2.tile(
                                    [P, c.SQ], F32, name="ps_m2", tag="ps_m2"
                                )
                                for kp in range(nhalf // 2):
                                    nc.tensor.matmul(
                                        ps, wm2t[:, j2, 2 * kp : 2 * kp + 2, :],
                                        HT[:, 2 * kp : 2 * kp + 2, :],
                                        start=(kp == 0),
                                        stop=(kp == nhalf // 2 - 1),
                                        perf_mode=DR,
                                    )
                                if half == 0:
                                    nc.vector.tensor_scalar_mul(
                                        OUT_ACC[:, j, :], ps, WSI
                                    )
                                else:
                                    nc.vector.scalar_tensor_tensor(
                                        out=OUT_ACC[:, j, :], in0=ps,
                                        scalar=WSI, in1=OUT_ACC[:, j, :],
                                        op0=OP.mult, op1=OP.add,
                                    )

            with tc.tile_pool(name="p3o", bufs=1) as p3o, \
                 tc.tile_pool(name="ps_gx", bufs=2, space="PSUM") as psg:
                OUT = p3o.tile([P, c.CH, c.SQ], F32)
                wg3 = gate_wtile(8)
                for j in range(c.CH):
                    gj = ada_gate_one(8, wg3, j, psg, WSI)
                    t = tw()
                    nc.vector.scalar_tensor_tensor(
                        out=t, in0=OUT_ACC[:, j, :], scalar=BM2[:, j : j + 1],
                        in1=gj, op0=OP.add, op1=OP.mult,
                    )
                    xrj = XC2[:, j, :]
                    nc.vector.tensor_add(OUT[:, j, :], t, xrj)
                    nc.sync.dma_start(out=outv[:, j, :], in_=OUT[:, j, :])

    nc.compile()
    return nc


# =======================================================
# Host side
# =======================================================

def host_prep(cfg: Cfg, inputs: dict):
    c = cfg
    f32 = np.float32

    q_x = np.asarray(inputs["q_x"], f32)
    h_content = np.asarray(inputs["h_content"], f32)
    h_obs = np.asarray(inputs["h_obs"], f32)
    t_cond = np.asarray(inputs["t_cond"], f32)
    M_QQ = np.asarray(inputs["M_QQ"], f32)
    M_hyb = np.asarray(inputs["M_hyb"], f32)
    w_ln_self = np.asarray(inputs["w_ln_self"], f32)
    w_qkv = np.asarray(inputs["w_qkv"], f32)
    w_self_out = np.asarray(inputs["w_self_out"], f32)
    w_ln_cross = np.asarray(inputs["w_ln_cross"], f32)
    w_ln_mem = np.asarray(inputs["w_ln_mem"], f32)
    w_qproj = np.asarray(inputs["w_qproj"], f32)
    w_kvproj = np.asarray(inputs["w_kvproj"], f32)
    w_cross_out = np.asarray(inputs["w_cross_out"], f32)
    w_ln_mlp = np.asarray(inputs["w_ln_mlp"], f32)
    w_mlp1 = np.asarray(inputs["w_mlp1"], f32)
    b_mlp1 = np.asarray(inputs["b_mlp1"], f32)
    w_mlp2 = np.asarray(inputs["w_mlp2"], f32)
    b_mlp2 = np.asarray(inputs["b_mlp2"], f32)
    w_ada = np.asarray(inputs["w_ada"], f32)
    b_ada = np.asarray(inputs["b_ada"], f32)

    D, N, HD, SQ = c.D, c.N, c.HD, c.SQ

    wada9 = w_ada[: 9 * D].copy()
    bada9 = b_ada[: 9 * D].copy()
    for q, wl in ((1, w_ln_self), (4, w_ln_cross), (7, w_ln_mlp)):
        wada9[q * D : (q + 1) * D] *= wl[:, None]
        bada9[q * D : (q + 1) * D] = wl * (1.0 + b_ada[q * D : (q + 1) * D])
    wadaT = np.ascontiguousarray(wada9.T)
    bada_u_h = np.ascontiguousarray(bada9.reshape(9 * c.CH, P).T)
    bada_h = np.ascontiguousarray(bada_u_h * WS)

    def bfc(a):
        return np.ascontiguousarray(a, dtype=BF)

    def f8c(a):
        return np.ascontiguousarray(
            np.clip(np.asarray(a, f32) * WS, -240.0, 240.0), dtype=F8H
        )

    def f8a(a):
        return np.ascontiguousarray(
            np.clip(np.asarray(a, f32), -240.0, 240.0), dtype=F8H
        )

    wqkvT = np.ascontiguousarray(w_qkv.T)
    wkv_eff = w_kvproj * w_ln_mem[None, :]
    wkvT = np.ascontiguousarray(wkv_eff.T)
    # pre-swizzled bf16 weight layouts ([P, blocks..., free])
    wada_s_h = f8c(wadaT.reshape(2, P, 9, 8, P).transpose(1, 2, 3, 0, 4))
    wqkv_qk_h = f8c(
        wqkvT[:, : 2 * D].reshape(8, P, 2, 8, P).transpose(1, 2, 3, 0, 4)
    )
    wqkv_v_h = f8c(
        wqkvT[:, 2 * D :].reshape(4, 2, P, 2, 512).transpose(2, 3, 0, 1, 4)
    )
    wself_o_h = f8c(w_self_out.T.reshape(8, P, 8, P).transpose(1, 2, 0, 3))
    wq_c_h = f8c(w_qproj.T.reshape(8, P, 8, P).transpose(1, 2, 0, 3))
    wkv_k_h = f8c(wkvT[:, :D].reshape(8, P, 8, P).transpose(1, 2, 0, 3))
    wkv_v_h = f8c(
        wkvT[:, D:].reshape(4, 2, P, 2, 512).transpose(2, 3, 0, 1, 4)
    )
    wcross_o_h = f8c(w_cross_out.T.reshape(8, P, 8, P).transpose(1, 2, 0, 3))
    wm1_s_h = f8c(w_mlp1.T.reshape(8, P, 32, P).transpose(1, 2, 0, 3))
    wm2_s_h = f8c(w_mlp2.T.reshape(2, 16, P, 8, P).transpose(2, 0, 3, 1, 4))
    bm1_h = np.ascontiguousarray(b_mlp1.reshape(c.DHC, P).T)
    bm2_h = np.ascontiguousarray(b_mlp2.reshape(c.CH, P).T)

    HH = c.HH
    selb_h = np.zeros((2 * HH, HH, P), np.float32)
    selb2_h = np.zeros((2 * HH, HH, 64), np.float32)
    for hp in range(HH):
        selb_h[2 * hp, hp, :64] = 1.0
        selb_h[2 * hp + 1, hp, 64:] = 1.0
        selb2_h[2 * hp + 1, hp, :] = 1.0
    selb_h = bfc(selb_h)
    selb2_h = bfc(selb2_h)
    ident_h = bfc(np.eye(P, dtype=np.float32))
    identd_h = f8a(np.stack([np.eye(P, dtype=np.float32)] * 2, 0)
               .transpose(1, 0, 2))
    sh32_h = np.zeros((P, P), np.float32)
    for i in range(P):
        sh32_h[i ^ 32, i] = 1.0
    sh32_h = bfc(sh32_h)

    # rowsums for the folded memory layernorm
    wsum = wkv_eff.sum(1).astype(f32)          # [2D]
    swk_h = np.ascontiguousarray(wsum[:D].reshape(c.HH, P).T)
    wsumv_h = np.ascontiguousarray(wsum[D:][None, :])

    pos = np.arange(N, dtype=f32)
    inv = (10000.0 ** (-np.arange(0, HD, 2, dtype=f32) / HD)).astype(f32)
    freqs = pos[:, None] * inv[None, :]
    cos64 = np.concatenate([np.cos(freqs), np.cos(freqs)], 1)
    s_sgn = np.concatenate([-np.sin(freqs), np.sin(freqs)], 1)
    c_pair = np.ascontiguousarray(np.tile(cos64.T, (2, 1)).astype(f32))
    s_pair = np.ascontiguousarray(np.tile(s_sgn.T, (2, 1)).astype(f32))
    scale = f32(1.0 / np.sqrt(HD))

    in_maps = []
    for b in range(c.B):
        xb = q_x[b]
        mu_x = xb.mean(-1).astype(f32)
        rs_x = (1.0 / np.sqrt(xb.var(-1) + c.eps)).astype(f32)
        mem = np.concatenate([h_content[b], h_obs[b]], 0)
        mu_m = mem.mean(-1).astype(f32)
        rs_m = (1.0 / np.sqrt(mem.var(-1) + c.eps)).astype(f32)
        mrs_m = (mu_m * rs_m).astype(f32)
        rs_cols_h = np.ascontiguousarray((rs_m / WS).reshape(2 * c.KK, P).T)
        mrs_cols_h = np.ascontiguousarray(mrs_m.reshape(2 * c.KK, P).T)
        mTQQ = np.maximum(M_QQ[b].T, -100.0).astype(f32)
        mThyb = np.maximum(M_hyb[b].T, -100.0).astype(f32)

        hc_s_h = f8a(
            h_content[b].T.reshape(8, P, 2, 512).transpose(1, 2, 0, 3)
        )
        ho_s_h = f8a(h_obs[b].T.reshape(8, P, 2, 512).transpose(1, 2, 0, 3))
        for s in range(2):
            own = np.arange(s * SQ, (s + 1) * SQ)
            rest = np.concatenate(
                [np.arange(0, s * SQ), np.arange((s + 1) * SQ, N)]
            )
            perm = np.concatenate([own, rest]).astype(np.int64)

            def swm(m):  # [N, SQ] mask -> [P, 8, 2, SQ] fp8, slot-1 zero
                t = m.reshape(8, P, SQ).transpose(1, 0, 2)
                z = np.zeros((P, 8, 2, SQ), np.float32)
                z[:, :, 0, :] = t
                return f8a(z)

            xhb = ((xb - mu_x[:, None]) * rs_x[:, None]).astype(f32)
            im = {
                "x_s": bfc(
                    xb.T[:, perm].reshape(8, P, N).transpose(1, 0, 2)
                ),
                "xh_s": bfc(
                    xhb.T[:, perm].reshape(8, P, N).transpose(1, 0, 2)
                ),
                "tc_s": f8a(
                    t_cond[b].T[:, perm].reshape(2, P, N).transpose(1, 0, 2)
                ),
                "hc_s": hc_s_h, "ho_s": ho_s_h,
                "wada_s": wada_s_h, "wqkv_qk": wqkv_qk_h,
                "wqkv_v": wqkv_v_h, "wself_o": wself_o_h,
                "wq_c": wq_c_h, "wkv_k": wkv_k_h, "wkv_v": wkv_v_h,
                "wcross_o": wcross_o_h, "wm1_s": wm1_s_h, "wm2_s": wm2_s_h,
                "bada": bada_h, "bada_u": bada_u_h,
                "bm1": bm1_h, "bm2": bm2_h,
                "cqt": bfc(c_pair[:, perm[:SQ]] * scale),
                "sqt": bfc(s_pair[:, perm[:SQ]] * scale),
                "ckts": bfc(c_pair[:, perm]),
                "skts": bfc(s_pair[:, perm]),
                "cktm": bfc(c_pair), "sktm": bfc(s_pair),
                "mself_s": swm(mTQQ[perm][:, perm[:SQ]]),
                "mhc_s": swm(mThyb[:N][:, perm[:SQ]]),
                "mho_s": swm(mThyb[N:][:, perm[:SQ]]),
                "la_self": bfc(rs_x[perm][None, :]),
                "lb_self": bfc(mu_x[perm][None, :]),
                "la_mc": bfc((rs_m / WS)[:N][None, :]),
                "lb_mc": bfc(mrs_m[:N][None, :]),
                "la_mo": bfc((rs_m / WS)[N:][None, :]),
                "lb_mo": bfc(mrs_m[N:][None, :]),
                "swk": swk_h, "wsumv": wsumv_h,
                "rs_cols": rs_cols_h, "mrs_cols": mrs_cols_h,
                "selb": selb_h, "selb2": selb2_h,
                "ident": ident_h, "sh32": sh32_h, "identd": identd_h,
            }
            in_maps.append(im)
    return in_maps


_PROGRAM_CACHE = {}


def get_program(cfg: Cfg):
    key = (cfg.N, cfg.D, cfg.H)
    if key not in _PROGRAM_CACHE:
        _PROGRAM_CACHE[key] = build_program(cfg)
    return _PROGRAM_CACHE[key]


def assemble(cfg: Cfg, results):
    c = cfg
    out = np.zeros((c.B, c.N, c.D), np.float32)
    for b in range(c.B):
        for s in range(2):
            o = results[2 * b + s]["out"]
            out[b, s * c.SQ : (s + 1) * c.SQ, :] = o.T
    return out


def kernel(**inputs) -> np.ndarray:
    cfg = Cfg(mini=False)
    nc = get_program(cfg)
    in_maps = host_prep(cfg, inputs)
    res = bass_utils.run_bass_kernel_spmd(
        nc, in_maps, core_ids=list(range(cfg.n_cores)), trace=False
    )
    return assemble(cfg, res.results)

